# revision 17
# baseline (speedup 1.0000x reference)
"""Trainium2 Bass kernel for nn_CCG_46273977647541.

Reference pipeline per batch (B=8 -> one NeuronCore each, no cross-core
communication): LayerNorm -> NxN cosine similarity -> density row-sum ->
argmax row as cluster center -> 256->64 projection + relu.

The NxN similarity is never materialized.  With ln_w==1, ln_b==0 (the
spec's deterministic fills) the density factorizes exactly through the
CENTERED rows xc_n = x_n - mu_n:

  u_n       = xc_n / |xc_n|,   |xc_n| = sqrt(C*var_n)
  density_n = u_n . sum_m u_m = q_n * (xc_n . S),  S = sum_m q_m xc_m
  q_n       = rsqrt(C*var_n)

Centering cancels out of the dots entirely: with t1 = sum_m q_m x_m over
the RAW rows,

  density_n = q_n * (x_n . t1  -  mu_n * sum(t1))

because xc_n . 1 = 0.  So the kernel never materializes centered data:

  DMA   x is loaded f32->bf16 with the cast done INLINE by the SDMA
        engines (SWDGE dtype-cast path, nc.gpsimd.dma_start).  This
        removes the entire per-tile cast pass from the compute engines;
        SBUF holds only the 2MB bf16 copy.
  DVE   paired-tile bn_stats ([P,2,256] -> [P,2,6]) + half-merges -> mu,
        var, q; then most of the 32 dot tiles (STT+accum vs broadcast t1).
  ACT   sqrt's; the remaining dot tiles via the square expansion
        x.t1 = (sum(x+t1)^2 - (C var + C mu^2) - sum t1^2)/2 over
        PE-built z=x+t1 PSUM pairs.
  PE    warmup chain (HAM clock-gate release), t1 matmuls (lhsT = q
        column, rhs = raw bf16 tile), z pairs, argmax transposes,
        center gather (+ mu correction via a ones-tile matmul), and the
        projection against a pre-transposed proj_w.

The center row is x-hat_j* = r_j* (x_j* - mu_j*) with r ~= sqrt(C)*q
(the +eps inside r is a 5e-6 relative perturbation; dropped).  The mu
correction of the gather is folded into a second accumulating matmul
with an all-ones rhs tile.

Numerics: bf16 data/matmuls with f32 accumulation, f32 stats and q.
Measured density error vs exact f32 on the spec inputs ~0.06 against a
minimum top-2 gap of 0.26; end-to-end relative error ~2e-3 (gate 2e-2).

Infrastructure notes: this walrus build accepts only ONE semaphore wait
per engine instruction and rejects some custom ISA ops; _split_multi_waits
post-processes the BIR JSON to hoist extra waits onto EventSemaphore
carriers and neutralize non-fatal SeqAsserts.
"""

import sys

sys.path.insert(0, "/opt/trn_rl_repo")

from contextlib import ExitStack

import numpy as np

import concourse.bass as bass
import concourse.tile as tile
from concourse import mybir
from concourse.bass_utils import run_bass_kernel_spmd
from concourse.tile import add_dep_helper

F32 = mybir.dt.float32
BF16 = mybir.dt.bfloat16
AX = mybir.AxisListType
OP = mybir.AluOpType
ACT = mybir.ActivationFunctionType


def _split_multi_waits(bir_json: bytes) -> bytes:
    """This walrus build accepts at most one semaphore wait per engine
    instruction.  Tile can emit several; hoist all but the last onto
    dedicated EventSemaphore carriers placed immediately before the
    instruction (same engine stream, so semantics are preserved --
    the block order is a topological order of the dep graph)."""
    import json as _json

    bir = _json.loads(bir_json)
    n = 0
    for fn in bir["functions"]:
        for bb in fn["blocks"]:
            new = []
            for inst in bb["instructions"]:
                if inst.get("op_name") == "SeqAssert":
                    inst = {
                        "debug": inst.get("debug", 0),
                        "engine": inst["engine"],
                        "ins": [],
                        "outs": [],
                        "name": inst["name"],
                        "opcode": "EventSemaphore",
                        "sync_info": inst.get("sync_info")
                        or {"on_update": [], "on_wait": []},
                    }
                si = inst.get("sync_info")
                waits = (si or {}).get("on_wait") or []
                if len(waits) > 1:
                    for w in waits[:-1]:
                        n += 1
                        new.append(
                            {
                                "debug": inst.get("debug", 0),
                                "engine": inst["engine"],
                                "ins": [],
                                "outs": [],
                                "name": f"antsplitw-{n}",
                                "opcode": "EventSemaphore",
                                "sync_info": {"on_update": [], "on_wait": [w]},
                            }
                        )
                    si["on_wait"] = [waits[-1]]
                new.append(inst)
            bb["instructions"] = new
    return _json.dumps(bir).encode()


def _install_wait_splitter():
    from concourse import bass_utils as _bu
    from concourse import bass2jax as _b2j

    if getattr(_bu, "_ant_wait_splitter", False):
        return
    _orig = _bu.compile_bir_kernel

    def _patched(bir_json, tmpdir, neff_name="file.neff"):
        return _orig(_split_multi_waits(bir_json), tmpdir, neff_name)

    _bu.compile_bir_kernel = _patched
    _bu._ant_wait_splitter = True
    if getattr(_b2j, "compile_bir_kernel", None) is _orig:
        _b2j.compile_bir_kernel = _patched


_install_wait_splitter()

B, N, C, CR = 8, 4096, 256, 64
P = 128
NT = N // P  # 32 row tiles per core
LN_EPS = 1e-5

_CACHE: dict = {}


def _build_nc() -> bass.Bass:
    nc = bass.Bass(enable_asserts=False)
    x_d = nc.declare_dram_parameter("x", [N, C], F32, isOutput=False)
    pw_d = nc.declare_dram_parameter("proj_w", [CR, C], F32, isOutput=False)
    pb_d = nc.declare_dram_parameter("proj_b", [CR], F32, isOutput=False)
    out_d = nc.declare_dram_parameter("out", [CR], F32, isOutput=True)

    with ExitStack() as ctx:
        tc = ctx.enter_context(tile.TileContext(nc))
        small = ctx.enter_context(tc.tile_pool(name="small", bufs=1))
        scrp = ctx.enter_context(tc.tile_pool(name="scr", bufs=6))
        psum = ctx.enter_context(tc.tile_pool(name="ps", bufs=1, space="PSUM"))
        zpool = ctx.enter_context(tc.tile_pool(name="z", bufs=2, space="PSUM"))

        # Row n of this core's batch lives at (partition n//NT, tile n%NT):
        # partition-major so each DMA descriptor reads contiguous DRAM.
        xb16 = small.tile([P, NT, C], BF16)
        NF = 4  # leading tiles loaded f32 on the SP HWDGE ring (fast start)
        xstage = small.tile([P, NF, C], F32)
        ST6 = small.tile([P, NT, 6], F32)  # per-tile even/odd half stats
        MSM2 = small.tile([P, NT, 2], F32)
        DD = small.tile([P, NT], F32)
        D2 = small.tile([P, NT], F32)
        DH = small.tile([P, NT], F32)
        M2C = small.tile([P, NT], F32)
        VA = small.tile([P, NT], F32)
        MU = small.tile([P, NT], F32)
        MU2 = small.tile([P, NT], F32)
        CV = small.tile([P, NT], F32)
        QS = small.tile([P, NT], F32)
        QQ = small.tile([P, NT], F32)
        QQb = small.tile([P, NT], BF16)
        XS = small.tile([P, NT], F32)
        XSQ = small.tile([P, NT], F32)
        CORR = small.tile([P, NT], F32)
        CORR2 = small.tile([P, NT], F32)
        TMPD = small.tile([P, NT], F32)
        DEN = small.tile([P, NT], F32)
        MASK = small.tile([P, NT], F32)
        MASKP = small.tile([P, NT], F32)
        W1 = small.tile([P, NT], F32)
        scrj = small.tile([P, NT], F32)
        IOTAJ = small.tile([P, NT], F32)
        dmax = small.tile([P, 1], F32)
        JIDX = small.tile([P, 1], F32)
        JIDX16 = small.tile([P, 1], BF16)
        pm16 = small.tile([P, 1], BF16)
        j32 = small.tile([1, 1], mybir.dt.int32)
        gm1 = small.tile([1, 1], F32)
        w1sel = small.tile([P, 1], F32)
        w1sel16 = small.tile([P, 1], BF16)
        w1mu = small.tile([P, 1], F32)
        nw1mu = small.tile([P, 1], F32)
        nw1mu16 = small.tile([P, 1], BF16)
        T1row = small.tile([1, 2 * C], BF16)
        T1b = small.tile([P, C], BF16)
        st1 = small.tile([1, 1], F32)
        nst1 = small.tile([1, 1], F32)
        ssq_scr = small.tile([1, C], BF16)
        ssS1 = small.tile([1, 1], F32)
        s1row = small.tile([1, C], F32)
        pw_sb = small.tile([CR, C], F32)
        pw16 = small.tile([CR, C], BF16)
        pwT0 = small.tile([P, CR], BF16)
        pwT1 = small.tile([P, CR], BF16)
        pb_row = small.tile([1, CR], F32)
        cen16 = small.tile([1, C], BF16)
        cencol = small.tile([P, 2], BF16)
        o_row = small.tile([1, CR], F32)
        warm = small.tile([1, 1], F32)
        ones_sb = small.tile([1, P], F32)
        ones16 = small.tile([1, P], BF16)
        wdum16 = small.tile([1, P], BF16)
        ones16C = small.tile([P, C], BF16)
        id_sb = small.tile([P, P], F32)
        Id16 = small.tile([P, P], BF16)
        onesPf = small.tile([P, P], F32)
        onesP16 = small.tile([P, P], BF16)
        ji32 = small.tile([P, NT], mybir.dt.int32)

        t1_ps = psum.tile([1, C], F32)
        wup_ps = psum.tile([P, P], F32, tag="wup")
        dmy_ps = psum.tile([1, 1], F32, tag="dmy")
        sb_ps = psum.tile([P, C], F32, tag="sb")
        pwt_ps = psum.tile([P, 2 * CR], BF16, tag="cen")

        xv = x_d[:, :].rearrange("(p j) c -> p j c", p=P)

        # ---- x DMA: first NF tiles f32 on the SP HWDGE ring (starts the
        # moment SP wakes; no SWDGE warmup in front) with the bf16 casts
        # done by the otherwise-idle ACT engine; the rest via SWDGE
        # (gpsimd) with the f32->bf16 cast done inline by the SDMA
        # engines.  Last chunks are single tiles to shorten the critical
        # tail after the final byte lands.
        nc.sync.dma_start(out=xstage[:, 0:2, :], in_=xv[:, 0:2, :])
        nc.sync.dma_start(out=xstage[:, 2:4, :], in_=xv[:, 2:4, :])
        CBND = [4, 8, 12, 16, 20, 24, 28, 30, 31, 32]
        for c in range(len(CBND) - 1):
            sl = slice(CBND[c], CBND[c + 1])
            nc.gpsimd.dma_start(out=xb16[:, sl, :], in_=xv[:, sl, :])
        # pw/pb on the SP ring behind the x head-chunks
        nc.sync.dma_start(out=pw_sb, in_=pw_d[:, :])
        nc.sync.dma_start(out=pb_row, in_=pb_d[None, :])

        # ---- Constants ----
        nc.vector.memset(warm, 1.0)
        nc.vector.memset(ones_sb, 1.0)
        nc.vector.memset(ones16, 1.0)
        nc.vector.memset(wdum16, 0.0)
        nc.vector.memset(ones16C, 1.0)
        # identity matrices + iota column built on the (post-DMA-issue)
        # Pool queue: affine_select picks in_ where j - p == 0
        nc.gpsimd.memset(onesPf, 1.0)
        nc.gpsimd.memset(onesP16, 1.0)
        nc.gpsimd.affine_select(
            out=id_sb, in_=onesPf, pattern=[[1, P]], compare_op=OP.is_equal,
            fill=0.0, base=0, channel_multiplier=-1,
        )
        nc.gpsimd.affine_select(
            out=Id16, in_=onesP16, pattern=[[1, P]], compare_op=OP.is_equal,
            fill=0.0, base=0, channel_multiplier=-1,
        )
        nc.gpsimd.iota(ji32, pattern=[[1, NT]], base=0, channel_multiplier=0)
        nc.gpsimd.tensor_copy(IOTAJ, ji32)

        # ACT table load (Sqrt) early
        nc.scalar.activation(out=warm, in_=warm, func=ACT.Sqrt)

        # ---- PE warmup: release the HAM clock gate (~3.4us of sustained
        # activity -> 1.2GHz cold to 2.4GHz warm) before the t1-chain.
        wu = nc.tensor.matmul(
            wup_ps[:, :], ones16[0:1, :], wdum16[0:1, :], start=True, stop=False
        )
        for _ in range(38):
            wu = nc.tensor.matmul(
                wup_ps[:, :], ones16[0:1, :], wdum16[0:1, :], start=False, stop=False
            )
        nc.tensor.matmul(
            wup_ps[:, :], ones16[0:1, :], wdum16[0:1, :], start=False, stop=True
        )

        # ---- pw pre-transpose (idle-time): cast + 2 PE transposes ----
        nc.scalar.copy(out=pw16, in_=pw_sb)
        nc.tensor.transpose(pwt_ps[:, 0:CR], pw16[0:CR, 0:P], Id16[0:CR, 0:CR])
        nc.tensor.transpose(pwt_ps[:, CR : 2 * CR], pw16[0:CR, P:C], Id16[0:CR, 0:CR])
        nc.vector.tensor_copy(pwT0, pwt_ps[:, 0:CR])
        nc.vector.tensor_copy(pwT1, pwt_ps[:, CR : 2 * CR])

        # ---- Phase 1: stats (DVE) + half-merges (Pool) + q (ACT/DVE) +
        # t1 chain (PE), pipelined chunk by chunk ----
        def _merge(eng, sl):
            # mu = (me+mo)/2 ; var = (M2e+M2o)/C + ((me-mo)/2)^2
            eng.tensor_add(MSM2[:, sl, :], ST6[:, sl, 1:3], ST6[:, sl, 4:6])
            eng.tensor_sub(DD[:, sl], ST6[:, sl, 1], ST6[:, sl, 4])
            eng.tensor_scalar_mul(DH[:, sl], DD[:, sl], 0.5)
            eng.tensor_mul(D2[:, sl], DH[:, sl], DH[:, sl])
            eng.tensor_scalar_mul(M2C[:, sl], MSM2[:, sl, 1], 1.0 / C)
            eng.tensor_add(VA[:, sl], M2C[:, sl], D2[:, sl])
            eng.tensor_scalar_mul(MU[:, sl], MSM2[:, sl, 0], 0.5)

        def _qchain(g0, g1):
            sl = slice(g0, g1)
            nc.scalar.activation(
                out=QS[:, sl], in_=VA[:, sl], func=ACT.Sqrt, scale=float(C)
            )
            nc.vector.reciprocal(out=QQ[:, sl], in_=QS[:, sl])
            nc.scalar.copy(out=QQb[:, sl], in_=QQ[:, sl])
            dmy = nc.tensor.matmul(
                dmy_ps[:, :], QQb[:, g0 : g0 + 1], QQb[:, g0 : g0 + 1],
                start=True, stop=True,
            )
            for j in range(g0, g1):
                mm1 = nc.tensor.matmul(
                    t1_ps[:, :], QQb[:, j : j + 1], xb16[:, j, :],
                    start=(j == 0), stop=(j == NT - 1),
                )
                add_dep_helper(mm1.ins, dmy.ins, False, "pe-prejoin")

        # stats for the f32 head tiles run straight off the staging buffer
        # (no cast dependency); ACT casts them into xb16 in parallel.
        for h in range(NF):
            nc.vector.bn_stats(out=ST6[:, h, :], in_=xstage[:, h, :])
            nc.scalar.copy(out=xb16[:, h, :], in_=xstage[:, h, :])
        _merge(nc.gpsimd, slice(0, NF))
        _qchain(0, NF)
        # SWDGE-cast tiles: stats in DMA order; merges on Pool except the
        # last small group (DVE, to cut the cross-engine tail latency)
        MGRP = [(4, 12, nc.gpsimd), (12, 20, nc.gpsimd), (20, 28, nc.gpsimd),
                (28, 32, nc.vector)]
        for g0, g1, eng in MGRP:
            for h in range(g0, g1):
                nc.vector.bn_stats(out=ST6[:, h, :], in_=xb16[:, h, :])
            _merge(eng, slice(g0, g1))
            _qchain(g0, g1)
        # square-path correction (Pool, off the critical path):
        #   CORR = (C/2) * (var + mu^2)   [= 0.5*(C var + C mu^2)]
        NSQ = 11
        slq = slice(0, NSQ)
        nc.gpsimd.tensor_mul(MU2[:, slq], MU[:, slq], MU[:, slq])
        nc.gpsimd.tensor_add(CV[:, slq], VA[:, slq], MU2[:, slq])
        nc.gpsimd.tensor_scalar_mul(CORR[:, slq], CV[:, slq], float(C) * 0.5)

        # ---- t1 finalize + broadcast ----
        nc.scalar.copy(out=T1row[0:1, 0:C], in_=t1_ps[0:1, :])
        nc.tensor.matmul(
            sb_ps[:, :], ones16[0:1, :], T1row[0:1, 0:C], start=True, stop=True
        )
        nc.vector.tensor_copy(T1b, sb_ps[:, :])
        # sum(t1) on ACT (only needed after the dots)
        nc.scalar.activation(
            out=s1row[0:1, :], in_=t1_ps[0:1, :], func=ACT.Identity,
            accum_out=st1[0:1, 0:1],
        )

        # ---- Phase 2: per-row dot x_n . t1 ----
        ssb_ps = psum.tile([P, 1], F32, tag="mx")
        st1b_ps = psum.tile([P, 1], F32, tag="sb")
        for j in range(NSQ, NT):
            scr = scrp.tile([P, C], BF16, tag="scr")
            st = nc.vector.scalar_tensor_tensor(
                out=scr, in0=xb16[:, j, :], scalar=1.0, in1=T1b,
                op0=OP.mult, op1=OP.mult, accum_out=XS[:, j : j + 1],
            )
            if j == NSQ:
                # z-path-only DVE work deferred past dot 0
                nc.vector.tensor_copy(T1row[0:1, C : 2 * C], t1_ps[0:1, :])
                nc.vector.scalar_tensor_tensor(
                    out=ssq_scr, in0=T1row[0:1, 0:C], scalar=1.0,
                    in1=T1row[0:1, 0:C], op0=OP.mult, op1=OP.mult,
                    accum_out=ssS1,
                )
                nc.vector.tensor_scalar_mul(ssS1, ssS1, 0.5)
                nc.vector.tensor_scalar_mul(nst1, st1, -1.0)
            if j == NSQ + 1:
                # consumers emitted AFTER their deferred inputs: z pairs
                # (gating the ACT squares), the sum-t1^2 and -sum(t1)
                # broadcasts
                for k in range(0, NSQ, 2):
                    k1 = min(k + 2, NSQ)
                    zp = zpool.tile([P, 2 * C], F32, tag="z")
                    nc.tensor.matmul(
                        zp[:, 0 : (k1 - k) * C], Id16[:, :],
                        xb16[:, k:k1, :], start=True, stop=False,
                    )
                    nc.tensor.matmul(
                        zp[:, 0 : (k1 - k) * C], ones16[0:1, :],
                        T1row[0:1, 0 : (k1 - k) * C], start=False, stop=True,
                    )
                    for t in range(k1 - k):
                        sqs = scrp.tile([P, C], BF16, tag="sqr")
                        nc.scalar.activation(
                            out=sqs, in_=zp[:, t * C : (t + 1) * C],
                            func=ACT.Square,
                            accum_out=XSQ[:, k + t : k + t + 1],
                        )
                nc.tensor.matmul(
                    ssb_ps[:, :], ones_sb[0:1, :], ssS1[0:1, 0:1],
                    start=True, stop=True,
                )
                nc.tensor.matmul(
                    st1b_ps[:, :], ones_sb[0:1, :], nst1[0:1, 0:1],
                    start=True, stop=True,
                )
            if j == NSQ + 3:
                # fold the 0.5*sum(t1)^2 term into CORR while dots run
                nc.vector.tensor_scalar(
                    out=CORR2[:, 0:NSQ], in0=CORR[:, 0:NSQ],
                    scalar1=ssb_ps[:, 0:1], scalar2=None, op0=OP.add,
                )
            if j in (15, 19, 23, 27, 31):
                # keep-warm: a paced dummy matmul (gated on this dot) so
                # the PE never sees a full 3.4us idle window
                kw = nc.tensor.matmul(
                    dmy_ps[:, :], QQb[:, 0:1], QQb[:, 0:1],
                    start=True, stop=True,
                )
                add_dep_helper(kw.ins, st.ins, False, "keepwarm")
        # finalize the square-trick columns: XS = 0.5*XSQ - CORR2
        nc.vector.scalar_tensor_tensor(
            out=XS[:, 0:NSQ], in0=XSQ[:, 0:NSQ], scalar=0.5, in1=CORR2[:, 0:NSQ],
            op0=OP.mult, op1=OP.subtract,
        )

        # density = q * (x.t1 - mu*sum(t1)):  (MU * (-st1)) + XS, then * QQ
        nc.vector.scalar_tensor_tensor(
            out=TMPD, in0=MU, scalar=st1b_ps[:, 0:1], in1=XS,
            op0=OP.mult, op1=OP.add,
        )
        nc.vector.tensor_mul(DEN, TMPD, QQ)

        # ---- Phase 3: global argmax ----
        # Per-partition max + argmax-index; global max via one transpose;
        # j* = <winner-partition one-hot, per-partition argmax index> on PE
        # (avoids any partition-offset reads of the transposed row).
        nc.vector.reduce_max(out=dmax, in_=DEN, axis=AX.X)
        nc.vector.tensor_scalar(
            out=MASKP, in0=DEN, scalar1=dmax[:, 0:1], scalar2=None, op0=OP.is_equal
        )
        nc.vector.scalar_tensor_tensor(
            out=scrj, in0=MASKP, scalar=1.0, in1=IOTAJ,
            op0=OP.mult, op1=OP.mult, accum_out=JIDX,
        )
        nc.vector.tensor_copy(JIDX16, JIDX)
        tr_ps = psum.tile([1, P], F32, tag="wup")
        nc.tensor.transpose(tr_ps[:, :], dmax[:, 0:1], id_sb[:, :])
        nc.vector.reduce_max(out=gm1, in_=tr_ps[0:1, :], axis=AX.X)
        gmax_ps = psum.tile([P, 1], F32, tag="mx")
        nc.tensor.matmul(
            gmax_ps[:, :], ones_sb[0:1, :], gm1[0:1, 0:1], start=True, stop=True
        )
        nc.vector.tensor_scalar(
            out=pm16, in0=dmax, scalar1=gmax_ps[:, 0:1], scalar2=None,
            op0=OP.is_equal,
        )
        js_ps = psum.tile([1, 1], F32, tag="dmy")
        nc.tensor.matmul(
            js_ps[:, :], pm16[:, 0:1], JIDX16[:, 0:1], start=True, stop=True
        )
        with nc.allow_low_precision(reason="exact small-int index value"):
            nc.vector.tensor_copy(j32, js_ps[0:1, 0:1])
        jv = nc.tensor.value_load(j32[0:1, 0:1])
        nc.vector.tensor_scalar(
            out=MASK, in0=DEN, scalar1=gmax_ps[:, 0:1], scalar2=None, op0=OP.is_equal
        )
        # w1 = mask * r, r = sqrt(C)*q  (the +eps in r is a 5e-6 rel
        # perturbation of the output row; dropped)
        nc.vector.scalar_tensor_tensor(
            out=W1, in0=MASK, scalar=float(np.sqrt(C)), in1=QQ,
            op0=OP.mult, op1=OP.mult,
        )
        nc.vector.reduce_sum(out=w1sel, in_=W1, axis=AX.X)
        nc.vector.tensor_copy(w1sel16, w1sel)
        # w2 = sum_j W1*MU (only the winner row survives the mask)
        nc.vector.scalar_tensor_tensor(
            out=scrj, in0=W1, scalar=1.0, in1=MU,
            op0=OP.mult, op1=OP.mult, accum_out=w1mu,
        )
        nc.vector.tensor_scalar_mul(nw1mu, w1mu, -1.0)
        nc.vector.tensor_copy(nw1mu16, nw1mu)

        # ---- Phase 4: center row = r*(x_j* - mu_j*) ----
        cc_ps = psum.tile([1, C], F32, tag="mx")
        nc.tensor.matmul(
            cc_ps[:, :], w1sel16[:, 0:1], xb16[:, bass.ds(jv, 1), :],
            start=True, stop=False,
        )
        nc.tensor.matmul(
            cc_ps[:, :], nw1mu16[:, 0:1], ones16C[:, :],
            start=False, stop=True,
        )
        nc.scalar.copy(out=cen16, in_=cc_ps[0:1, :])

        # ---- Phase 5: out = relu(proj_w @ center + proj_b) via pwT ----
        # PSUM bf16 writes must be 4B aligned: put the two halves at
        # element offsets 0 and 2 of a [P, 4] tile.
        ccol_ps = psum.tile([P, 4], BF16, tag="cen")
        nc.tensor.transpose(ccol_ps[:, 0:1], cen16[0:1, 0:P], Id16[0:1, 0:1])
        nc.tensor.transpose(ccol_ps[:, 2:3], cen16[0:1, P:C], Id16[0:1, 0:1])
        nc.vector.tensor_copy(cencol[:, 0:1], ccol_ps[:, 0:1])
        nc.vector.tensor_copy(cencol[:, 1:2], ccol_ps[:, 2:3])
        o_ps = psum.tile([1, CR], F32, tag="cen")
        nc.tensor.matmul(
            o_ps[:, :], cencol[:, 0:1], pwT0[:, :], start=True, stop=False
        )
        nc.tensor.matmul(
            o_ps[:, :], cencol[:, 1:2], pwT1[:, :], start=False, stop=True
        )
        nc.vector.tensor_add(o_row, o_ps[0:1, :], pb_row[0:1, :])
        nc.vector.tensor_scalar_max(out=o_row, in0=o_row, scalar1=0.0)
        nc.sync.dma_start(out=out_d[None, :], in_=o_row)

    return nc


def _get_nc() -> bass.Bass:
    if "nc" not in _CACHE:
        _CACHE["nc"] = _build_nc()
    return _CACHE["nc"]


def _ensure_ntff_hook():
    """The image's antenv package lacks axon_hooks; shim it so
    run_bass_kernel_spmd(trace=True) can reach the NTFF profiler."""
    import types

    if "antenv.axon_hooks" in sys.modules:
        return
    m = types.ModuleType("antenv.axon_hooks")
    _hook = [None]
    m.set_axon_ntff_profile_hook = lambda h: _hook.__setitem__(0, h)
    m.get_axon_ntff_profile_hook = lambda: _hook[0]
    sys.modules["antenv.axon_hooks"] = m
    try:
        import antenv

        antenv.axon_hooks = m
        from trn_agent_boot.trn_boot import _ntff_profile_via_ctypes

        m.set_axon_ntff_profile_hook(
            _ntff_profile_via_ctypes("/opt/axon/libaxon_pjrt.so")
        )
    except Exception:
        pass


def _run(x, proj_w, proj_b, trace=False):
    if trace:
        _ensure_ntff_hook()
    nc = _get_nc()
    in_maps = [
        {
            "x": np.ascontiguousarray(x[b], dtype=np.float32),
            "proj_w": np.ascontiguousarray(proj_w, dtype=np.float32),
            "proj_b": np.ascontiguousarray(proj_b, dtype=np.float32),
        }
        for b in range(B)
    ]
    res = run_bass_kernel_spmd(nc, in_maps, list(range(B)), trace=trace)
    out = np.stack([res.results[b]["out"].reshape(1, CR) for b in range(B)])
    return out.astype(np.float32), res


def kernel(x, ln_w, ln_b, proj_w, proj_b):
    x = np.asarray(x)
    ln_w = np.asarray(ln_w)
    ln_b = np.asarray(ln_b)
    proj_w = np.asarray(proj_w)
    proj_b = np.asarray(proj_b)
    if not (np.allclose(ln_w, 1.0) and np.allclose(ln_b, 0.0)):
        # General ln_w/ln_b fallback (never hit with the spec's fills: ones/zeros).
        return _kernel_numpy(x, ln_w, ln_b, proj_w, proj_b)
    out, _ = _run(x, proj_w, proj_b, trace=False)
    return out


def _kernel_numpy(x, ln_w, ln_b, proj_w, proj_b):
    x = x.astype(np.float32)
    mu = x.mean(-1, keepdims=True)
    var = x.var(-1, keepdims=True)
    xn = (x - mu) / np.sqrt(var + LN_EPS) * ln_w + ln_b
    nrm = np.linalg.norm(xn, axis=-1, keepdims=True)
    out = []
    for b in range(x.shape[0]):
        cos = (xn[b] @ xn[b].T) / (nrm[b] @ nrm[b].T + 1e-8)
        den = cos.sum(-1)
        mask = (den == den.max()).astype(np.float32)[:, None]
        center = (xn[b] * mask).sum(0)
        out.append(np.maximum(proj_w @ center + proj_b, 0.0))
    return np.stack(out)[:, None, :].astype(np.float32)


# revision 26
# speedup vs baseline: 1.0341x; 1.0341x over previous
"""Trainium2 Bass kernel for nn_CCG_46273977647541.

Reference pipeline per batch (B=8 -> one NeuronCore each, no cross-core
communication): LayerNorm -> NxN cosine similarity -> density row-sum ->
argmax row as cluster center -> 256->64 projection + relu.

The NxN similarity is never materialized.  With ln_w==1, ln_b==0 (the
spec's deterministic fills) the density factorizes exactly through the
CENTERED rows xc_n = x_n - mu_n:

  u_n       = xc_n / |xc_n|,   |xc_n| = sqrt(C*var_n)
  density_n = u_n . sum_m u_m = q_n * (xc_n . S),  S = sum_m q_m xc_m
  q_n       = rsqrt(C*var_n)

Centering cancels out of the dots entirely: with t1 = sum_m q_m x_m over
the RAW rows,

  density_n = q_n * (x_n . t1  -  mu_n * sum(t1))

because xc_n . 1 = 0.  So the kernel never materializes centered data:

  DMA   x is loaded f32->bf16 with the cast done INLINE by the SDMA
        engines (SWDGE dtype-cast path, nc.gpsimd.dma_start).  This
        removes the entire per-tile cast pass from the compute engines;
        SBUF holds only the 2MB bf16 copy.
  DVE   paired-tile bn_stats ([P,2,256] -> [P,2,6]) + half-merges -> mu,
        var, q; then most of the 32 dot tiles (STT+accum vs broadcast t1).
  ACT   sqrt's; the remaining dot tiles via the square expansion
        x.t1 = (sum(x+t1)^2 - (C var + C mu^2) - sum t1^2)/2 over
        PE-built z=x+t1 PSUM pairs.
  PE    warmup chain (HAM clock-gate release), t1 matmuls (lhsT = q
        column, rhs = raw bf16 tile), z pairs, argmax transposes,
        center gather (+ mu correction via a ones-tile matmul), and the
        projection against a pre-transposed proj_w.

The center row is x-hat_j* = r_j* (x_j* - mu_j*) with r ~= sqrt(C)*q
(the +eps inside r is a 5e-6 relative perturbation; dropped).  The mu
correction of the gather is folded into a second accumulating matmul
with an all-ones rhs tile.

Numerics: bf16 data/matmuls with f32 accumulation, f32 stats and q.
Measured density error vs exact f32 on the spec inputs ~0.06 against a
minimum top-2 gap of 0.26; end-to-end relative error ~2e-3 (gate 2e-2).

Infrastructure notes: this walrus build accepts only ONE semaphore wait
per engine instruction and rejects some custom ISA ops; _split_multi_waits
post-processes the BIR JSON to hoist extra waits onto EventSemaphore
carriers and neutralize non-fatal SeqAsserts.
"""

import sys

sys.path.insert(0, "/opt/trn_rl_repo")

from contextlib import ExitStack

import numpy as np

import concourse.bass as bass
import concourse.tile as tile
from concourse import mybir
from concourse.bass_utils import run_bass_kernel_spmd
from concourse.tile import add_dep_helper

F32 = mybir.dt.float32
BF16 = mybir.dt.bfloat16
AX = mybir.AxisListType
OP = mybir.AluOpType
ACT = mybir.ActivationFunctionType


def _split_multi_waits(bir_json: bytes) -> bytes:
    """This walrus build accepts at most one semaphore wait per engine
    instruction.  Tile can emit several; hoist all but the last onto
    dedicated EventSemaphore carriers placed immediately before the
    instruction (same engine stream, so semantics are preserved --
    the block order is a topological order of the dep graph)."""
    import json as _json

    bir = _json.loads(bir_json)
    n = 0
    for fn in bir["functions"]:
        for bb in fn["blocks"]:
            new = []
            for inst in bb["instructions"]:
                if inst.get("op_name") == "SeqAssert":
                    inst = {
                        "debug": inst.get("debug", 0),
                        "engine": inst["engine"],
                        "ins": [],
                        "outs": [],
                        "name": inst["name"],
                        "opcode": "EventSemaphore",
                        "sync_info": inst.get("sync_info")
                        or {"on_update": [], "on_wait": []},
                    }
                si = inst.get("sync_info")
                waits = (si or {}).get("on_wait") or []
                if len(waits) > 1:
                    for w in waits[:-1]:
                        n += 1
                        new.append(
                            {
                                "debug": inst.get("debug", 0),
                                "engine": inst["engine"],
                                "ins": [],
                                "outs": [],
                                "name": f"antsplitw-{n}",
                                "opcode": "EventSemaphore",
                                "sync_info": {"on_update": [], "on_wait": [w]},
                            }
                        )
                    si["on_wait"] = [waits[-1]]
                new.append(inst)
            bb["instructions"] = new
    return _json.dumps(bir).encode()


def _install_wait_splitter():
    from concourse import bass_utils as _bu
    from concourse import bass2jax as _b2j

    if getattr(_bu, "_ant_wait_splitter", False):
        return
    _orig = _bu.compile_bir_kernel

    def _patched(bir_json, tmpdir, neff_name="file.neff"):
        return _orig(_split_multi_waits(bir_json), tmpdir, neff_name)

    _bu.compile_bir_kernel = _patched
    _bu._ant_wait_splitter = True
    if getattr(_b2j, "compile_bir_kernel", None) is _orig:
        _b2j.compile_bir_kernel = _patched


_install_wait_splitter()

B, N, C, CR = 8, 4096, 256, 64
P = 128
NT = N // P  # 32 row tiles per core
LN_EPS = 1e-5

_CACHE: dict = {}


def _build_nc() -> bass.Bass:
    nc = bass.Bass(enable_asserts=False)
    x_d = nc.declare_dram_parameter("x", [N, C], F32, isOutput=False)
    pw_d = nc.declare_dram_parameter("proj_w", [CR, C], F32, isOutput=False)
    pb_d = nc.declare_dram_parameter("proj_b", [CR], F32, isOutput=False)
    out_d = nc.declare_dram_parameter("out", [CR], F32, isOutput=True)

    with ExitStack() as ctx:
        tc = ctx.enter_context(tile.TileContext(nc))
        small = ctx.enter_context(tc.tile_pool(name="small", bufs=1))
        scrp = ctx.enter_context(tc.tile_pool(name="scr", bufs=6))
        psum = ctx.enter_context(tc.tile_pool(name="ps", bufs=1, space="PSUM"))
        zpool = ctx.enter_context(tc.tile_pool(name="z", bufs=2, space="PSUM"))

        # Row n of this core's batch lives at (partition n//NT, tile n%NT):
        # partition-major so each DMA descriptor reads contiguous DRAM.
        xb16 = small.tile([P, NT, C], BF16)
        ST6 = small.tile([P, NT, 6], F32)  # per-tile even/odd half stats
        MSM2 = small.tile([P, NT, 2], F32)
        DD = small.tile([P, NT], F32)
        D2 = small.tile([P, NT], F32)
        DH = small.tile([P, NT], F32)
        M2C = small.tile([P, NT], F32)
        VA = small.tile([P, NT], F32)
        MU = small.tile([P, NT], F32)
        MU2 = small.tile([P, NT], F32)
        CV = small.tile([P, NT], F32)
        QS = small.tile([P, NT], F32)
        QQ = small.tile([P, NT], F32)
        QQb = small.tile([P, NT], BF16)
        XS = small.tile([P, NT], F32)
        XSQ = small.tile([P, NT], F32)
        CORR = small.tile([P, NT], F32)
        CORR2 = small.tile([P, NT], F32)
        TMPD = small.tile([P, NT], F32)
        DEN = small.tile([P, NT], F32)
        MASK = small.tile([P, NT], F32)
        MASKP = small.tile([P, NT], F32)
        W1 = small.tile([P, NT], F32)
        scrj = small.tile([P, NT], F32)
        IOTAJ = small.tile([P, NT], F32)
        ONESN = small.tile([P, NT], F32)
        dmax = small.tile([P, 1], F32)
        JIDX = small.tile([P, 1], F32)
        JIDX16 = small.tile([P, 1], BF16)
        pm16 = small.tile([P, 1], BF16)
        j32 = small.tile([1, 1], mybir.dt.int32)
        gm1 = small.tile([1, 1], F32)
        w1sel = small.tile([P, 1], F32)
        w1sel16 = small.tile([P, 1], BF16)
        w1mu = small.tile([P, 1], F32)
        nw1mu = small.tile([P, 1], F32)
        nw1mu16 = small.tile([P, 1], BF16)
        T1row = small.tile([1, 2 * C], BF16)
        T1b = small.tile([P, C], BF16)
        st1 = small.tile([1, 1], F32)
        nst1 = small.tile([1, 1], F32)
        ssq_scr = small.tile([1, C], BF16)
        ssS1 = small.tile([1, 1], F32)
        s1row = small.tile([1, C], F32)
        pw_sb = small.tile([CR, C], F32)
        pw16 = small.tile([CR, C], BF16)
        pwT0 = small.tile([P, CR], BF16)
        pwT1 = small.tile([P, CR], BF16)
        pb_row = small.tile([1, CR], F32)
        cen16 = small.tile([1, C], BF16)
        cencol = small.tile([P, 2], BF16)
        o_row = small.tile([1, CR], F32)
        warm = small.tile([1, 1], F32)
        ones_sb = small.tile([1, P], F32)
        ones16 = small.tile([1, P], BF16)
        wdum16 = small.tile([1, P], BF16)
        ones16C = small.tile([P, C], BF16)
        id_sb = small.tile([P, P], F32)
        Id16 = small.tile([P, P], BF16)
        onesPf = small.tile([P, P], F32)
        onesP16 = small.tile([P, P], BF16)
        ji32 = small.tile([P, NT], mybir.dt.int32)

        t1_ps = psum.tile([1, C], F32)
        wup_ps = psum.tile([P, P], F32, tag="wup")
        dmy_ps = psum.tile([1, 1], F32, tag="dmy")
        sb_ps = psum.tile([P, C], F32, tag="sb")
        pwt_ps = psum.tile([P, 2 * CR], BF16, tag="cen")

        xv = x_d[:, :].rearrange("(p j) c -> p j c", p=P)

        # ---- x DMA: SWDGE (gpsimd) with the f32->bf16 cast done inline
        # by the SDMA engines; issued before all other Pool work.  The
        # last chunk is short so the post-load critical tail is short.
        CBND = [0, 2, 6, 10, 14, 18, 22, 26, 30, 32]
        for c in range(len(CBND) - 1):
            sl = slice(CBND[c], CBND[c + 1])
            nc.gpsimd.dma_start(out=xb16[:, sl, :], in_=xv[:, sl, :])
        # pw/pb on the (idle) SP HWDGE ring
        nc.sync.dma_start(out=pw_sb, in_=pw_d[:, :])
        nc.sync.dma_start(out=pb_row, in_=pb_d[None, :])

        # ---- Constants (DVE; engines otherwise idle pre-load) ----
        nc.vector.memset(warm, 1.0)
        nc.vector.memset(ones_sb, 1.0)
        nc.vector.memset(ones16, 1.0)
        nc.vector.memset(wdum16, 0.0)
        nc.vector.memset(ones16C, 1.0)
        nc.vector.memset(ONESN, 1.0)

        # ACT table load (Sqrt) early
        nc.scalar.activation(out=warm, in_=warm, func=ACT.Sqrt)

        # ---- PE warmup: release the HAM clock gate (~3.4us of sustained
        # activity -> 1.2GHz cold to 2.4GHz warm) before the t1-chain.
        wu = nc.tensor.matmul(
            wup_ps[:, :], ones16[0:1, :], wdum16[0:1, :], start=True, stop=False
        )
        for _ in range(38):
            wu = nc.tensor.matmul(
                wup_ps[:, :], ones16[0:1, :], wdum16[0:1, :], start=False, stop=False
            )
        nc.tensor.matmul(
            wup_ps[:, :], ones16[0:1, :], wdum16[0:1, :], start=False, stop=True
        )

        # ---- Phase 1 ----
        # DVE runs a PURE bn_stats stream (no other DVE work may sit in
        # its in-order queue during the load, or a stalled op blocks all
        # later stats).  Pool does the half-merges and reciprocals for
        # the first groups; the last group's merge/recip run on DVE
        # right after its final bn_stats (lower cross-engine latency).
        # ACT does the sqrt and the f32->bf16 q cast; PE accumulates t1.
        def _merge(eng, sl):
            # mu = (me+mo)/2 ; var = (M2e+M2o)/C + ((me-mo)/2)^2
            eng.tensor_add(MSM2[:, sl, :], ST6[:, sl, 1:3], ST6[:, sl, 4:6])
            eng.tensor_sub(DD[:, sl], ST6[:, sl, 1], ST6[:, sl, 4])
            eng.tensor_scalar_mul(DH[:, sl], DD[:, sl], 0.5)
            eng.tensor_mul(D2[:, sl], DH[:, sl], DH[:, sl])
            eng.tensor_scalar_mul(M2C[:, sl], MSM2[:, sl, 1], 1.0 / C)
            eng.tensor_add(VA[:, sl], M2C[:, sl], D2[:, sl])
            eng.tensor_scalar_mul(MU[:, sl], MSM2[:, sl, 0], 0.5)

        def _merge_dve(sl):
            nc.vector.tensor_add(MSM2[:, sl, :], ST6[:, sl, 1:3], ST6[:, sl, 4:6])
            nc.vector.tensor_sub(DD[:, sl], ST6[:, sl, 1], ST6[:, sl, 4])
            nc.vector.scalar_tensor_tensor(
                out=D2[:, sl], in0=DD[:, sl], scalar=0.25, in1=DD[:, sl],
                op0=OP.mult, op1=OP.mult,
            )
            nc.vector.scalar_tensor_tensor(
                out=VA[:, sl], in0=MSM2[:, sl, 1], scalar=1.0 / C, in1=D2[:, sl],
                op0=OP.mult, op1=OP.add,
            )
            nc.vector.tensor_scalar_mul(MU[:, sl], MSM2[:, sl, 0], 0.5)

        def _t1mm(g0, g1):
            dmy = nc.tensor.matmul(
                dmy_ps[:, :], QQb[:, g0 : g0 + 1], QQb[:, g0 : g0 + 1],
                start=True, stop=True,
            )
            for j in range(g0, g1):
                mm1 = nc.tensor.matmul(
                    t1_ps[:, :], QQb[:, j : j + 1], xb16[:, j, :],
                    start=(j == 0), stop=(j == NT - 1),
                )
                add_dep_helper(mm1.ins, dmy.ins, False, "pe-prejoin")

        NSQ = 11
        GA, GB = 12, 26  # merge groups [0,GA) + [GA,GB) on Pool, [GB,32) DVE
        # Emission order IS the per-engine queue order AND the dependency
        # order (Tile tracks deps by program order).  The DVE stream must
        # stay a near-pure bn_stats run: each (DVE-only) reciprocal is
        # spliced in a few stats AFTER its producers were emitted, so by
        # the time the queue reaches it the (Pool merge -> ACT sqrt)
        # chain has finished and nothing stalls.
        for h in range(0, GA):
            nc.vector.bn_stats(out=ST6[:, h, :], in_=xb16[:, h, :])
        _merge(nc.gpsimd, slice(0, GA))
        nc.scalar.activation(
            out=QS[:, 0:GA], in_=VA[:, 0:GA], func=ACT.Sqrt, scale=float(C)
        )
        for h in range(GA, GA + 6):
            nc.vector.bn_stats(out=ST6[:, h, :], in_=xb16[:, h, :])
        nc.vector.reciprocal(out=QQ[:, 0:GA], in_=QS[:, 0:GA])
        nc.scalar.copy(out=QQb[:, 0:GA], in_=QQ[:, 0:GA])
        _t1mm(0, GA)
        for h in range(GA + 6, GB):
            nc.vector.bn_stats(out=ST6[:, h, :], in_=xb16[:, h, :])
        _merge(nc.gpsimd, slice(GA, GB))
        nc.scalar.activation(
            out=QS[:, GA:GB], in_=VA[:, GA:GB], func=ACT.Sqrt, scale=float(C)
        )
        # square-path correction (Pool): CORR = (C/2) * (var + mu^2)
        slq = slice(0, NSQ)
        nc.gpsimd.tensor_mul(MU2[:, slq], MU[:, slq], MU[:, slq])
        nc.gpsimd.tensor_add(CV[:, slq], VA[:, slq], MU2[:, slq])
        nc.gpsimd.tensor_scalar_mul(CORR[:, slq], CV[:, slq], float(C) * 0.5)
        for h in range(GB, NT):
            nc.vector.bn_stats(out=ST6[:, h, :], in_=xb16[:, h, :])
        nc.vector.reciprocal(out=QQ[:, GA:GB], in_=QS[:, GA:GB])
        nc.scalar.copy(out=QQb[:, GA:GB], in_=QQ[:, GA:GB])
        _t1mm(GA, GB)
        # last group: merge + q on DVE/ACT right behind bn_stats 31
        _merge_dve(slice(GB, NT))
        nc.scalar.activation(
            out=QS[:, GB:NT], in_=VA[:, GB:NT], func=ACT.Sqrt, scale=float(C)
        )
        nc.vector.reciprocal(out=QQ[:, GB:NT], in_=QS[:, GB:NT])
        nc.scalar.copy(out=QQb[:, GB:NT], in_=QQ[:, GB:NT])
        _t1mm(GB, NT)
        # identity matrices + iota column (Pool, needed from the dot
        # phase onward): affine_select picks in_ where j - p == 0
        nc.gpsimd.memset(onesPf, 1.0)
        nc.gpsimd.memset(onesP16, 1.0)
        nc.gpsimd.affine_select(
            out=id_sb, in_=onesPf, pattern=[[1, P]], compare_op=OP.is_equal,
            fill=0.0, base=0, channel_multiplier=-1,
        )
        nc.gpsimd.affine_select(
            out=Id16, in_=onesP16, pattern=[[1, P]], compare_op=OP.is_equal,
            fill=0.0, base=0, channel_multiplier=-1,
        )
        nc.gpsimd.iota(ji32, pattern=[[1, NT]], base=0, channel_multiplier=0)
        nc.gpsimd.tensor_copy(IOTAJ, ji32)

        # ---- pw pre-transpose (idle-time): cast + 2 PE transposes ----
        nc.scalar.copy(out=pw16, in_=pw_sb)
        nc.tensor.transpose(pwt_ps[:, 0:CR], pw16[0:CR, 0:P], Id16[0:CR, 0:CR])
        nc.tensor.transpose(pwt_ps[:, CR : 2 * CR], pw16[0:CR, P:C], Id16[0:CR, 0:CR])

        # ---- t1 finalize + broadcast ----
        nc.scalar.copy(out=T1row[0:1, 0:C], in_=t1_ps[0:1, :])
        nc.tensor.matmul(
            sb_ps[:, :], ones16[0:1, :], T1row[0:1, 0:C], start=True, stop=True
        )
        nc.vector.tensor_copy(T1b, sb_ps[:, :])
        # sum(t1) on ACT (only needed after the dots)
        nc.scalar.activation(
            out=s1row[0:1, :], in_=t1_ps[0:1, :], func=ACT.Identity,
            accum_out=st1[0:1, 0:1],
        )

        # ---- Phase 2: per-row dot x_n . t1 ----
        ssb_ps = psum.tile([P, 1], F32, tag="mx")
        st1b_ps = psum.tile([P, 1], F32, tag="sb")
        for j in range(NSQ, NT):
            scr = scrp.tile([P, C], BF16, tag="scr")
            st = nc.vector.scalar_tensor_tensor(
                out=scr, in0=xb16[:, j, :], scalar=1.0, in1=T1b,
                op0=OP.mult, op1=OP.mult, accum_out=XS[:, j : j + 1],
            )
            if j == NSQ:
                # z-path-only DVE work deferred past dot 0
                nc.vector.tensor_copy(T1row[0:1, C : 2 * C], t1_ps[0:1, :])
                nc.vector.scalar_tensor_tensor(
                    out=ssq_scr, in0=T1row[0:1, 0:C], scalar=1.0,
                    in1=T1row[0:1, 0:C], op0=OP.mult, op1=OP.mult,
                    accum_out=ssS1,
                )
                nc.vector.tensor_scalar_mul(ssS1, ssS1, 0.5)
                nc.vector.tensor_scalar_mul(nst1, st1, -1.0)
            if j == NSQ + 1:
                # consumers emitted AFTER their deferred inputs: z pairs
                # (gating the ACT squares), the sum-t1^2 and -sum(t1)
                # broadcasts
                for k in range(0, NSQ, 2):
                    k1 = min(k + 2, NSQ)
                    zp = zpool.tile([P, 2 * C], F32, tag="z")
                    nc.tensor.matmul(
                        zp[:, 0 : (k1 - k) * C], Id16[:, :],
                        xb16[:, k:k1, :], start=True, stop=False,
                    )
                    nc.tensor.matmul(
                        zp[:, 0 : (k1 - k) * C], ones16[0:1, :],
                        T1row[0:1, 0 : (k1 - k) * C], start=False, stop=True,
                    )
                    for t in range(k1 - k):
                        sqs = scrp.tile([P, C], BF16, tag="sqr")
                        nc.scalar.activation(
                            out=sqs, in_=zp[:, t * C : (t + 1) * C],
                            func=ACT.Square,
                            accum_out=XSQ[:, k + t : k + t + 1],
                        )
                nc.tensor.matmul(
                    ssb_ps[:, :], ones_sb[0:1, :], ssS1[0:1, 0:1],
                    start=True, stop=True,
                )
                nc.tensor.matmul(
                    st1b_ps[:, :], ones_sb[0:1, :], nst1[0:1, 0:1],
                    start=True, stop=True,
                )
            if j == NSQ + 3:
                # fold the 0.5*sum(t1)^2 term into CORR while dots run
                nc.vector.tensor_scalar(
                    out=CORR2[:, 0:NSQ], in0=CORR[:, 0:NSQ],
                    scalar1=ssb_ps[:, 0:1], scalar2=None, op0=OP.add,
                )
            if j == NSQ + 5:
                # pw^T copyback (PSUM -> SBUF), needed only at the proj
                nc.vector.tensor_copy(pwT0, pwt_ps[:, 0:CR])
                nc.vector.tensor_copy(pwT1, pwt_ps[:, CR : 2 * CR])
            if j in (15, 19, 23, 27, 31):
                # keep-warm: a paced dummy matmul (gated on this dot) so
                # the PE never sees a full 3.4us idle window
                kw = nc.tensor.matmul(
                    dmy_ps[:, :], QQb[:, 0:1], QQb[:, 0:1],
                    start=True, stop=True,
                )
                add_dep_helper(kw.ins, st.ins, False, "keepwarm")
        # finalize the square-trick columns: XS = 0.5*XSQ - CORR2
        nc.vector.scalar_tensor_tensor(
            out=XS[:, 0:NSQ], in0=XSQ[:, 0:NSQ], scalar=0.5, in1=CORR2[:, 0:NSQ],
            op0=OP.mult, op1=OP.subtract,
        )

        # density = q * (x.t1 - mu*sum(t1)):  (MU * (-st1)) + XS, then * QQ
        nc.vector.scalar_tensor_tensor(
            out=TMPD, in0=MU, scalar=st1b_ps[:, 0:1], in1=XS,
            op0=OP.mult, op1=OP.add,
        )
        nc.vector.tensor_mul(DEN, TMPD, QQ)

        # ---- Phase 3: global argmax ----
        # Per-partition max + argmax-index; global max via one transpose;
        # j* = <winner-partition one-hot, per-partition argmax index> on PE
        # (avoids any partition-offset reads of the transposed row).
        nc.vector.reduce_max(out=dmax, in_=DEN, axis=AX.X)
        nc.vector.tensor_scalar(
            out=MASKP, in0=DEN, scalar1=dmax[:, 0:1], scalar2=None, op0=OP.is_equal
        )
        nc.vector.scalar_tensor_tensor(
            out=scrj, in0=MASKP, scalar=1.0, in1=IOTAJ,
            op0=OP.mult, op1=OP.mult, accum_out=JIDX,
        )
        nc.vector.tensor_copy(JIDX16, JIDX)
        tr_ps = psum.tile([1, P], F32, tag="wup")
        nc.tensor.transpose(tr_ps[:, :], dmax[:, 0:1], id_sb[:, :])
        nc.vector.reduce_max(out=gm1, in_=tr_ps[0:1, :], axis=AX.X)
        gmax_ps = psum.tile([P, 1], F32, tag="mx")
        nc.tensor.matmul(
            gmax_ps[:, :], ones_sb[0:1, :], gm1[0:1, 0:1], start=True, stop=True
        )
        nc.vector.tensor_scalar(
            out=pm16, in0=dmax, scalar1=gmax_ps[:, 0:1], scalar2=None,
            op0=OP.is_equal,
        )
        js_ps = psum.tile([1, 1], F32, tag="dmy")
        nc.tensor.matmul(
            js_ps[:, :], pm16[:, 0:1], JIDX16[:, 0:1], start=True, stop=True
        )
        with nc.allow_low_precision(reason="exact small-int index value"):
            nc.vector.tensor_copy(j32, js_ps[0:1, 0:1])
        jv = nc.tensor.value_load(j32[0:1, 0:1])
        nc.vector.tensor_scalar(
            out=MASK, in0=DEN, scalar1=gmax_ps[:, 0:1], scalar2=None, op0=OP.is_equal
        )
        # w1 = mask * r, r = sqrt(C)*q  (the +eps in r is a 5e-6 rel
        # perturbation of the output row; dropped)
        nc.vector.scalar_tensor_tensor(
            out=W1, in0=MASK, scalar=float(np.sqrt(C)), in1=QQ,
            op0=OP.mult, op1=OP.mult,
        )
        nc.vector.reduce_sum(out=w1sel, in_=W1, axis=AX.X)
        nc.vector.tensor_copy(w1sel16, w1sel)
        # w2 = sum_j W1*MU (only the winner row survives the mask)
        nc.vector.scalar_tensor_tensor(
            out=scrj, in0=W1, scalar=1.0, in1=MU,
            op0=OP.mult, op1=OP.mult, accum_out=w1mu,
        )
        nc.vector.tensor_scalar_mul(nw1mu, w1mu, -1.0)
        nc.vector.tensor_copy(nw1mu16, nw1mu)

        # ---- Phase 4: center row = r*(x_j* - mu_j*) ----
        cc_ps = psum.tile([1, C], F32, tag="mx")
        nc.tensor.matmul(
            cc_ps[:, :], w1sel16[:, 0:1], xb16[:, bass.ds(jv, 1), :],
            start=True, stop=False,
        )
        nc.tensor.matmul(
            cc_ps[:, :], nw1mu16[:, 0:1], ones16C[:, :],
            start=False, stop=True,
        )
        nc.scalar.copy(out=cen16, in_=cc_ps[0:1, :])

        # ---- Phase 5: out = relu(proj_w @ center + proj_b) via pwT ----
        # PSUM bf16 writes must be 4B aligned: put the two halves at
        # element offsets 0 and 2 of a [P, 4] tile.
        ccol_ps = psum.tile([P, 4], BF16, tag="cen")
        nc.tensor.transpose(ccol_ps[:, 0:1], cen16[0:1, 0:P], Id16[0:1, 0:1])
        nc.tensor.transpose(ccol_ps[:, 2:3], cen16[0:1, P:C], Id16[0:1, 0:1])
        nc.vector.tensor_copy(cencol[:, 0:1], ccol_ps[:, 0:1])
        nc.vector.tensor_copy(cencol[:, 1:2], ccol_ps[:, 2:3])
        o_ps = psum.tile([1, CR], F32, tag="cen")
        nc.tensor.matmul(
            o_ps[:, :], cencol[:, 0:1], pwT0[:, :], start=True, stop=False
        )
        nc.tensor.matmul(
            o_ps[:, :], cencol[:, 1:2], pwT1[:, :], start=False, stop=True
        )
        nc.vector.tensor_add(o_row, o_ps[0:1, :], pb_row[0:1, :])
        nc.vector.tensor_scalar_max(out=o_row, in0=o_row, scalar1=0.0)
        nc.sync.dma_start(out=out_d[None, :], in_=o_row)

    return nc


def _get_nc() -> bass.Bass:
    if "nc" not in _CACHE:
        _CACHE["nc"] = _build_nc()
    return _CACHE["nc"]


def _ensure_ntff_hook():
    """The image's antenv package lacks axon_hooks; shim it so
    run_bass_kernel_spmd(trace=True) can reach the NTFF profiler."""
    import types

    if "antenv.axon_hooks" in sys.modules:
        return
    m = types.ModuleType("antenv.axon_hooks")
    _hook = [None]
    m.set_axon_ntff_profile_hook = lambda h: _hook.__setitem__(0, h)
    m.get_axon_ntff_profile_hook = lambda: _hook[0]
    sys.modules["antenv.axon_hooks"] = m
    try:
        import antenv

        antenv.axon_hooks = m
        from trn_agent_boot.trn_boot import _ntff_profile_via_ctypes

        m.set_axon_ntff_profile_hook(
            _ntff_profile_via_ctypes("/opt/axon/libaxon_pjrt.so")
        )
    except Exception:
        pass


def _run(x, proj_w, proj_b, trace=False):
    if trace:
        _ensure_ntff_hook()
    nc = _get_nc()
    in_maps = [
        {
            "x": np.ascontiguousarray(x[b], dtype=np.float32),
            "proj_w": np.ascontiguousarray(proj_w, dtype=np.float32),
            "proj_b": np.ascontiguousarray(proj_b, dtype=np.float32),
        }
        for b in range(B)
    ]
    res = run_bass_kernel_spmd(nc, in_maps, list(range(B)), trace=trace)
    out = np.stack([res.results[b]["out"].reshape(1, CR) for b in range(B)])
    return out.astype(np.float32), res


def kernel(x, ln_w, ln_b, proj_w, proj_b):
    x = np.asarray(x)
    ln_w = np.asarray(ln_w)
    ln_b = np.asarray(ln_b)
    proj_w = np.asarray(proj_w)
    proj_b = np.asarray(proj_b)
    if not (np.allclose(ln_w, 1.0) and np.allclose(ln_b, 0.0)):
        # General ln_w/ln_b fallback (never hit with the spec's fills: ones/zeros).
        return _kernel_numpy(x, ln_w, ln_b, proj_w, proj_b)
    out, _ = _run(x, proj_w, proj_b, trace=False)
    return out


def _kernel_numpy(x, ln_w, ln_b, proj_w, proj_b):
    x = x.astype(np.float32)
    mu = x.mean(-1, keepdims=True)
    var = x.var(-1, keepdims=True)
    xn = (x - mu) / np.sqrt(var + LN_EPS) * ln_w + ln_b
    nrm = np.linalg.norm(xn, axis=-1, keepdims=True)
    out = []
    for b in range(x.shape[0]):
        cos = (xn[b] @ xn[b].T) / (nrm[b] @ nrm[b].T + 1e-8)
        den = cos.sum(-1)
        mask = (den == den.max()).astype(np.float32)[:, None]
        center = (xn[b] * mask).sum(0)
        out.append(np.maximum(proj_w @ center + proj_b, 0.0))
    return np.stack(out)[:, None, :].astype(np.float32)


# revision 30
# speedup vs baseline: 1.0604x; 1.0255x over previous
"""Trainium2 Bass kernel for nn_CCG_46273977647541.

Reference pipeline per batch (B=8 -> one NeuronCore each, no cross-core
communication): LayerNorm -> NxN cosine similarity -> density row-sum ->
argmax row as cluster center -> 256->64 projection + relu.

The NxN similarity is never materialized.  With ln_w==1, ln_b==0 (the
spec's deterministic fills) the density factorizes exactly through the
CENTERED rows xc_n = x_n - mu_n:

  u_n       = xc_n / |xc_n|,   |xc_n| = sqrt(C*var_n)
  density_n = u_n . sum_m u_m = q_n * (xc_n . S),  S = sum_m q_m xc_m
  q_n       = rsqrt(C*var_n)

Centering cancels out of the dots entirely: with t1 = sum_m q_m x_m over
the RAW rows,

  density_n = q_n * (x_n . t1  -  mu_n * sum(t1))

because xc_n . 1 = 0.  So the kernel never materializes centered data:

  DMA   x is loaded f32->bf16 with the cast done INLINE by the SDMA
        engines (SWDGE dtype-cast path, nc.gpsimd.dma_start).  This
        removes the entire per-tile cast pass from the compute engines;
        SBUF holds only the 2MB bf16 copy.
  DVE   paired-tile bn_stats ([P,2,256] -> [P,2,6]) + half-merges -> mu,
        var, q; then most of the 32 dot tiles (STT+accum vs broadcast t1).
  ACT   sqrt's; the remaining dot tiles via the square expansion
        x.t1 = (sum(x+t1)^2 - (C var + C mu^2) - sum t1^2)/2 over
        PE-built z=x+t1 PSUM pairs.
  PE    warmup chain (HAM clock-gate release), t1 matmuls (lhsT = q
        column, rhs = raw bf16 tile), z pairs, argmax transposes,
        center gather (+ mu correction via a ones-tile matmul), and the
        projection against a pre-transposed proj_w.

The center row is x-hat_j* = r_j* (x_j* - mu_j*) with r ~= sqrt(C)*q
(the +eps inside r is a 5e-6 relative perturbation; dropped).  The mu
correction of the gather is folded into a second accumulating matmul
with an all-ones rhs tile.

Numerics: bf16 data/matmuls with f32 accumulation, f32 stats and q.
Measured density error vs exact f32 on the spec inputs ~0.06 against a
minimum top-2 gap of 0.26; end-to-end relative error ~2e-3 (gate 2e-2).

Infrastructure notes: this walrus build accepts only ONE semaphore wait
per engine instruction and rejects some custom ISA ops; _split_multi_waits
post-processes the BIR JSON to hoist extra waits onto EventSemaphore
carriers and neutralize non-fatal SeqAsserts.
"""

import sys

sys.path.insert(0, "/opt/trn_rl_repo")

from contextlib import ExitStack

import numpy as np

import concourse.bass as bass
import concourse.tile as tile
from concourse import mybir
from concourse.bass_utils import run_bass_kernel_spmd
from concourse.tile import add_dep_helper

F32 = mybir.dt.float32
BF16 = mybir.dt.bfloat16
AX = mybir.AxisListType
OP = mybir.AluOpType
ACT = mybir.ActivationFunctionType


def _split_multi_waits(bir_json: bytes) -> bytes:
    """This walrus build accepts at most one semaphore wait per engine
    instruction.  Tile can emit several; hoist all but the last onto
    dedicated EventSemaphore carriers placed immediately before the
    instruction (same engine stream, so semantics are preserved --
    the block order is a topological order of the dep graph)."""
    import json as _json

    bir = _json.loads(bir_json)
    n = 0
    for fn in bir["functions"]:
        for bb in fn["blocks"]:
            new = []
            for inst in bb["instructions"]:
                if inst.get("op_name") == "SeqAssert":
                    inst = {
                        "debug": inst.get("debug", 0),
                        "engine": inst["engine"],
                        "ins": [],
                        "outs": [],
                        "name": inst["name"],
                        "opcode": "EventSemaphore",
                        "sync_info": inst.get("sync_info")
                        or {"on_update": [], "on_wait": []},
                    }
                si = inst.get("sync_info")
                waits = (si or {}).get("on_wait") or []
                if len(waits) > 1:
                    for w in waits[:-1]:
                        n += 1
                        new.append(
                            {
                                "debug": inst.get("debug", 0),
                                "engine": inst["engine"],
                                "ins": [],
                                "outs": [],
                                "name": f"antsplitw-{n}",
                                "opcode": "EventSemaphore",
                                "sync_info": {"on_update": [], "on_wait": [w]},
                            }
                        )
                    si["on_wait"] = [waits[-1]]
                new.append(inst)
            bb["instructions"] = new
    return _json.dumps(bir).encode()


def _install_wait_splitter():
    from concourse import bass_utils as _bu
    from concourse import bass2jax as _b2j

    if getattr(_bu, "_ant_wait_splitter", False):
        return
    _orig = _bu.compile_bir_kernel

    def _patched(bir_json, tmpdir, neff_name="file.neff"):
        return _orig(_split_multi_waits(bir_json), tmpdir, neff_name)

    _bu.compile_bir_kernel = _patched
    _bu._ant_wait_splitter = True
    if getattr(_b2j, "compile_bir_kernel", None) is _orig:
        _b2j.compile_bir_kernel = _patched


_install_wait_splitter()

B, N, C, CR = 8, 4096, 256, 64
P = 128
NT = N // P  # 32 row tiles per core
LN_EPS = 1e-5

_CACHE: dict = {}


def _build_nc() -> bass.Bass:
    nc = bass.Bass(enable_asserts=False)
    x_d = nc.declare_dram_parameter("x", [N, C], F32, isOutput=False)
    pw_d = nc.declare_dram_parameter("proj_w", [CR, C], F32, isOutput=False)
    pb_d = nc.declare_dram_parameter("proj_b", [CR], F32, isOutput=False)
    out_d = nc.declare_dram_parameter("out", [CR], F32, isOutput=True)

    with ExitStack() as ctx:
        tc = ctx.enter_context(tile.TileContext(nc))
        small = ctx.enter_context(tc.tile_pool(name="small", bufs=1))
        scrp = ctx.enter_context(tc.tile_pool(name="scr", bufs=6))
        psum = ctx.enter_context(tc.tile_pool(name="ps", bufs=1, space="PSUM"))
        zpool = ctx.enter_context(tc.tile_pool(name="z", bufs=2, space="PSUM"))

        # Row n of this core's batch lives at (partition n//NT, tile n%NT):
        # partition-major so each DMA descriptor reads contiguous DRAM.
        xb16 = small.tile([P, NT, C], BF16)
        ST6 = small.tile([P, NT, 6], F32)  # per-tile even/odd half stats
        MSM2 = small.tile([P, NT, 2], F32)
        DD = small.tile([P, NT], F32)
        D2 = small.tile([P, NT], F32)
        DH = small.tile([P, NT], F32)
        M2C = small.tile([P, NT], F32)
        VA = small.tile([P, NT], F32)
        MU = small.tile([P, NT], F32)
        MU2 = small.tile([P, NT], F32)
        CV = small.tile([P, NT], F32)
        QS = small.tile([P, NT], F32)
        QQ = small.tile([P, NT], F32)
        QQb = small.tile([P, NT], BF16)
        XS = small.tile([P, NT], F32)
        XSQ = small.tile([P, NT], F32)
        CORR = small.tile([P, NT], F32)
        CORR2 = small.tile([P, NT], F32)
        TMPD = small.tile([P, NT], F32)
        DEN = small.tile([P, NT], F32)
        MASK = small.tile([P, NT], F32)
        MASKP = small.tile([P, NT], F32)
        W1 = small.tile([P, NT], F32)
        scrj = small.tile([P, NT], F32)
        IOTAJ = small.tile([P, NT], F32)
        ONESN = small.tile([P, NT], F32)
        dmax = small.tile([P, 1], F32)
        JIDX = small.tile([P, 1], F32)
        JIDX16 = small.tile([P, 1], BF16)
        pm16 = small.tile([P, 1], BF16)
        j32 = small.tile([1, 1], mybir.dt.int32)
        gm1 = small.tile([1, 1], F32)
        w1sel = small.tile([P, 1], F32)
        w1sel16 = small.tile([P, 1], BF16)
        w1mu = small.tile([P, 1], F32)
        nw1mu = small.tile([P, 1], F32)
        nw1mu16 = small.tile([P, 1], BF16)
        T1row = small.tile([1, 2 * C], BF16)
        T1b = small.tile([P, C], BF16)
        st1 = small.tile([1, 1], F32)
        nst1 = small.tile([1, 1], F32)
        ssq_scr = small.tile([1, C], BF16)
        ssS1 = small.tile([1, 1], F32)
        s1row = small.tile([1, C], F32)
        pw_sb = small.tile([CR, C], F32)
        pw16 = small.tile([CR, C], BF16)
        pwT0 = small.tile([P, CR], BF16)
        pwT1 = small.tile([P, CR], BF16)
        pb_row = small.tile([1, CR], F32)
        cen16 = small.tile([1, C], BF16)
        cencol = small.tile([P, 2], BF16)
        o_row = small.tile([1, CR], F32)
        warm = small.tile([1, 1], F32)
        ones_sb = small.tile([1, P], F32)
        ones16 = small.tile([1, P], BF16)
        wdum16 = small.tile([1, P], BF16)
        ones16C = small.tile([P, C], BF16)
        id_sb = small.tile([P, P], F32)
        Id16 = small.tile([P, P], BF16)
        onesPf = small.tile([P, P], F32)
        onesP16 = small.tile([P, P], BF16)
        ji32 = small.tile([P, NT], mybir.dt.int32)

        t1_ps = psum.tile([1, C], F32)
        wup_ps = psum.tile([P, P], F32, tag="wup")
        dmy_ps = psum.tile([1, 1], F32, tag="dmy")
        sb_ps = psum.tile([P, C], F32, tag="sb")
        pwt_ps = psum.tile([P, 2 * CR], BF16, tag="cen")

        xv = x_d[:, :].rearrange("(p j) c -> p j c", p=P)

        # ---- x DMA: SWDGE (gpsimd) with the f32->bf16 cast done inline
        # by the SDMA engines; issued before all other Pool work.  The
        # tail chunks shrink to 1 tile: a chunk's data only becomes
        # visible ~1-2us (completion receipt) after its LAST byte, so
        # small late chunks cut the post-load stats latency.
        CBND = [0, 4, 8, 12, 16, 20, 24, 28, 30, 31, 32]
        for c in range(len(CBND) - 1):
            sl = slice(CBND[c], CBND[c + 1])
            nc.gpsimd.dma_start(out=xb16[:, sl, :], in_=xv[:, sl, :])
        # pw/pb on the (idle) SP HWDGE ring
        nc.sync.dma_start(out=pw_sb, in_=pw_d[:, :])
        nc.sync.dma_start(out=pb_row, in_=pb_d[None, :])

        # ---- Constants (DVE; engines otherwise idle pre-load) ----
        nc.vector.memset(warm, 1.0)
        nc.vector.memset(ones_sb, 1.0)
        nc.vector.memset(ones16, 1.0)
        nc.vector.memset(wdum16, 0.0)
        nc.vector.memset(ones16C, 1.0)
        nc.vector.memset(ONESN, 1.0)

        # ACT table load (Sqrt) early
        nc.scalar.activation(out=warm, in_=warm, func=ACT.Sqrt)

        # ---- PE warmup: release the HAM clock gate (~3.4us of sustained
        # activity -> 1.2GHz cold to 2.4GHz warm) before the t1-chain.
        wu = nc.tensor.matmul(
            wup_ps[:, :], ones16[0:1, :], wdum16[0:1, :], start=True, stop=False
        )
        for _ in range(38):
            wu = nc.tensor.matmul(
                wup_ps[:, :], ones16[0:1, :], wdum16[0:1, :], start=False, stop=False
            )
        nc.tensor.matmul(
            wup_ps[:, :], ones16[0:1, :], wdum16[0:1, :], start=False, stop=True
        )

        # ---- Phase 1 ----
        # DVE runs a PURE bn_stats stream (no other DVE work may sit in
        # its in-order queue during the load, or a stalled op blocks all
        # later stats).  Pool does the half-merges and reciprocals for
        # the first groups; the last group's merge/recip run on DVE
        # right after its final bn_stats (lower cross-engine latency).
        # ACT does the sqrt and the f32->bf16 q cast; PE accumulates t1.
        def _merge(eng, sl):
            # mu = (me+mo)/2 ; var = (M2e+M2o)/C + ((me-mo)/2)^2
            eng.tensor_add(MSM2[:, sl, :], ST6[:, sl, 1:3], ST6[:, sl, 4:6])
            eng.tensor_sub(DD[:, sl], ST6[:, sl, 1], ST6[:, sl, 4])
            eng.tensor_scalar_mul(DH[:, sl], DD[:, sl], 0.5)
            eng.tensor_mul(D2[:, sl], DH[:, sl], DH[:, sl])
            eng.tensor_scalar_mul(M2C[:, sl], MSM2[:, sl, 1], 1.0 / C)
            eng.tensor_add(VA[:, sl], M2C[:, sl], D2[:, sl])
            eng.tensor_scalar_mul(MU[:, sl], MSM2[:, sl, 0], 0.5)

        def _merge_dve(sl):
            nc.vector.tensor_add(MSM2[:, sl, :], ST6[:, sl, 1:3], ST6[:, sl, 4:6])
            nc.vector.tensor_sub(DD[:, sl], ST6[:, sl, 1], ST6[:, sl, 4])
            nc.vector.scalar_tensor_tensor(
                out=D2[:, sl], in0=DD[:, sl], scalar=0.25, in1=DD[:, sl],
                op0=OP.mult, op1=OP.mult,
            )
            nc.vector.scalar_tensor_tensor(
                out=VA[:, sl], in0=MSM2[:, sl, 1], scalar=1.0 / C, in1=D2[:, sl],
                op0=OP.mult, op1=OP.add,
            )
            nc.vector.tensor_scalar_mul(MU[:, sl], MSM2[:, sl, 0], 0.5)

        def _t1mm(g0, g1):
            dmy = nc.tensor.matmul(
                dmy_ps[:, :], QQb[:, g0 : g0 + 1], QQb[:, g0 : g0 + 1],
                start=True, stop=True,
            )
            for j in range(g0, g1):
                mm1 = nc.tensor.matmul(
                    t1_ps[:, :], QQb[:, j : j + 1], xb16[:, j, :],
                    start=(j == 0), stop=(j == NT - 1),
                )
                add_dep_helper(mm1.ins, dmy.ins, False, "pe-prejoin")

        NSQ = 11
        GA, GB = 12, 26  # merge groups: [0,GA) Pool, [GA,GB) + [GB,32) DVE
        # PE keep-warm through the load: one tiny matmul gated on each
        # chunk's data (reads one xb16 column) so the HAM clock gate
        # stays released and all later matmuls run at 2.4GHz.  Emitted
        # interleaved with the stats stream so they sit at matching
        # positions in the PE queue.
        KW_TILES = set(CBND[1:-1])

        def _bn(h):
            nc.vector.bn_stats(out=ST6[:, h, :], in_=xb16[:, h, :])
            if h in KW_TILES:
                nc.tensor.matmul(
                    dmy_ps[:, :], xb16[:, h, 0:1], xb16[:, h, 0:1],
                    start=True, stop=True,
                )

        # DVE: the bn_stats stream with merges/recips spliced into the
        # chunk-receipt wait gaps (each emitted AFTER its producers).
        for h in range(0, GA):
            _bn(h)
        _merge(nc.gpsimd, slice(0, GA))
        nc.scalar.activation(
            out=QS[:, 0:GA], in_=VA[:, 0:GA], func=ACT.Sqrt, scale=float(C)
        )
        for h in range(GA, GA + 6):
            _bn(h)
        nc.vector.reciprocal(out=QQ[:, 0:GA], in_=QS[:, 0:GA])
        nc.scalar.copy(out=QQb[:, 0:GA], in_=QQ[:, 0:GA])
        _t1mm(0, GA)
        for h in range(GA + 6, GB):
            _bn(h)
        _merge_dve(slice(GA, GB))
        nc.scalar.activation(
            out=QS[:, GA:GB], in_=VA[:, GA:GB], func=ACT.Sqrt, scale=float(C)
        )
        for h in range(GB, GB + 2):
            _bn(h)
        nc.vector.reciprocal(out=QQ[:, GA:GB], in_=QS[:, GA:GB])
        nc.scalar.copy(out=QQb[:, GA:GB], in_=QQ[:, GA:GB])
        _t1mm(GA, GB)
        for h in range(GB + 2, NT):
            _bn(h)
        # last group: merge + q on DVE/ACT right behind bn_stats 31
        _merge_dve(slice(GB, NT))
        nc.scalar.activation(
            out=QS[:, GB:NT], in_=VA[:, GB:NT], func=ACT.Sqrt, scale=float(C)
        )
        nc.vector.reciprocal(out=QQ[:, GB:NT], in_=QS[:, GB:NT])
        nc.scalar.copy(out=QQb[:, GB:NT], in_=QQ[:, GB:NT])
        _t1mm(GB, NT)
        # square-path correction (Pool): CORR = (C/2) * (var + mu^2)
        slq = slice(0, NSQ)
        nc.gpsimd.tensor_mul(MU2[:, slq], MU[:, slq], MU[:, slq])
        nc.gpsimd.tensor_add(CV[:, slq], VA[:, slq], MU2[:, slq])
        nc.gpsimd.tensor_scalar_mul(CORR[:, slq], CV[:, slq], float(C) * 0.5)
        # identity matrices + iota column (Pool, needed from the dot
        # phase onward): affine_select picks in_ where j - p == 0
        nc.gpsimd.memset(onesPf, 1.0)
        nc.gpsimd.memset(onesP16, 1.0)
        nc.gpsimd.affine_select(
            out=id_sb, in_=onesPf, pattern=[[1, P]], compare_op=OP.is_equal,
            fill=0.0, base=0, channel_multiplier=-1,
        )
        nc.gpsimd.affine_select(
            out=Id16, in_=onesP16, pattern=[[1, P]], compare_op=OP.is_equal,
            fill=0.0, base=0, channel_multiplier=-1,
        )
        nc.gpsimd.iota(ji32, pattern=[[1, NT]], base=0, channel_multiplier=0)
        nc.gpsimd.tensor_copy(IOTAJ, ji32)

        # ---- pw pre-transpose (idle-time): cast + 2 PE transposes ----
        nc.scalar.copy(out=pw16, in_=pw_sb)
        nc.tensor.transpose(pwt_ps[:, 0:CR], pw16[0:CR, 0:P], Id16[0:CR, 0:CR])
        nc.tensor.transpose(pwt_ps[:, CR : 2 * CR], pw16[0:CR, P:C], Id16[0:CR, 0:CR])

        # ---- t1 finalize + broadcast ----
        nc.scalar.copy(out=T1row[0:1, 0:C], in_=t1_ps[0:1, :])
        nc.tensor.matmul(
            sb_ps[:, :], ones16[0:1, :], T1row[0:1, 0:C], start=True, stop=True
        )
        nc.vector.tensor_copy(T1b, sb_ps[:, :])
        # sum(t1) on ACT (only needed after the dots)
        nc.scalar.activation(
            out=s1row[0:1, :], in_=t1_ps[0:1, :], func=ACT.Identity,
            accum_out=st1[0:1, 0:1],
        )

        # ---- Phase 2: per-row dot x_n . t1 ----
        ssb_ps = psum.tile([P, 1], F32, tag="mx")
        st1b_ps = psum.tile([P, 1], F32, tag="sb")
        for j in range(NSQ, NT):
            scr = scrp.tile([P, C], BF16, tag="scr")
            st = nc.vector.scalar_tensor_tensor(
                out=scr, in0=xb16[:, j, :], scalar=1.0, in1=T1b,
                op0=OP.mult, op1=OP.mult, accum_out=XS[:, j : j + 1],
            )
            if j == NSQ:
                # z-path-only DVE work deferred past dot 0
                nc.vector.tensor_copy(T1row[0:1, C : 2 * C], t1_ps[0:1, :])
                nc.vector.scalar_tensor_tensor(
                    out=ssq_scr, in0=T1row[0:1, 0:C], scalar=1.0,
                    in1=T1row[0:1, 0:C], op0=OP.mult, op1=OP.mult,
                    accum_out=ssS1,
                )
                nc.vector.tensor_scalar_mul(ssS1, ssS1, 0.5)
                nc.vector.tensor_scalar_mul(nst1, st1, -1.0)
            if j == NSQ + 1:
                # consumers emitted AFTER their deferred inputs: z pairs
                # (gating the ACT squares), the sum-t1^2 and -sum(t1)
                # broadcasts
                for k in range(0, NSQ, 2):
                    k1 = min(k + 2, NSQ)
                    zp = zpool.tile([P, 2 * C], F32, tag="z")
                    nc.tensor.matmul(
                        zp[:, 0 : (k1 - k) * C], Id16[:, :],
                        xb16[:, k:k1, :], start=True, stop=False,
                    )
                    nc.tensor.matmul(
                        zp[:, 0 : (k1 - k) * C], ones16[0:1, :],
                        T1row[0:1, 0 : (k1 - k) * C], start=False, stop=True,
                    )
                    for t in range(k1 - k):
                        sqs = scrp.tile([P, C], BF16, tag="sqr")
                        nc.scalar.activation(
                            out=sqs, in_=zp[:, t * C : (t + 1) * C],
                            func=ACT.Square,
                            accum_out=XSQ[:, k + t : k + t + 1],
                        )
                nc.tensor.matmul(
                    ssb_ps[:, :], ones_sb[0:1, :], ssS1[0:1, 0:1],
                    start=True, stop=True,
                )
                nc.tensor.matmul(
                    st1b_ps[:, :], ones_sb[0:1, :], nst1[0:1, 0:1],
                    start=True, stop=True,
                )
            if j == NSQ + 3:
                # fold the 0.5*sum(t1)^2 term into CORR while dots run
                nc.vector.tensor_scalar(
                    out=CORR2[:, 0:NSQ], in0=CORR[:, 0:NSQ],
                    scalar1=ssb_ps[:, 0:1], scalar2=None, op0=OP.add,
                )
            if j == NSQ + 5:
                # pw^T copyback (PSUM -> SBUF), needed only at the proj
                nc.vector.tensor_copy(pwT0, pwt_ps[:, 0:CR])
                nc.vector.tensor_copy(pwT1, pwt_ps[:, CR : 2 * CR])
            if j in (15, 19, 23, 27, 31):
                # keep-warm: a paced dummy matmul (gated on this dot) so
                # the PE never sees a full 3.4us idle window
                kw = nc.tensor.matmul(
                    dmy_ps[:, :], QQb[:, 0:1], QQb[:, 0:1],
                    start=True, stop=True,
                )
                add_dep_helper(kw.ins, st.ins, False, "keepwarm")
        # finalize the square-trick columns: XS = 0.5*XSQ - CORR2
        nc.vector.scalar_tensor_tensor(
            out=XS[:, 0:NSQ], in0=XSQ[:, 0:NSQ], scalar=0.5, in1=CORR2[:, 0:NSQ],
            op0=OP.mult, op1=OP.subtract,
        )

        # density = q * (x.t1 - mu*sum(t1)):  (MU * (-st1)) + XS, then * QQ
        nc.vector.scalar_tensor_tensor(
            out=TMPD, in0=MU, scalar=st1b_ps[:, 0:1], in1=XS,
            op0=OP.mult, op1=OP.add,
        )
        nc.vector.tensor_mul(DEN, TMPD, QQ)

        # ---- Phase 3: global argmax ----
        # Per-partition max + argmax-index; global max via one transpose;
        # j* = <winner-partition one-hot, per-partition argmax index> on PE
        # (avoids any partition-offset reads of the transposed row).
        nc.vector.reduce_max(out=dmax, in_=DEN, axis=AX.X)
        nc.vector.tensor_scalar(
            out=MASKP, in0=DEN, scalar1=dmax[:, 0:1], scalar2=None, op0=OP.is_equal
        )
        nc.vector.scalar_tensor_tensor(
            out=scrj, in0=MASKP, scalar=1.0, in1=IOTAJ,
            op0=OP.mult, op1=OP.mult, accum_out=JIDX,
        )
        nc.vector.tensor_copy(JIDX16, JIDX)
        tr_ps = psum.tile([1, P], F32, tag="wup")
        nc.tensor.transpose(tr_ps[:, :], dmax[:, 0:1], id_sb[:, :])
        nc.vector.reduce_max(out=gm1, in_=tr_ps[0:1, :], axis=AX.X)
        gmax_ps = psum.tile([P, 1], F32, tag="mx")
        nc.tensor.matmul(
            gmax_ps[:, :], ones_sb[0:1, :], gm1[0:1, 0:1], start=True, stop=True
        )
        nc.vector.tensor_scalar(
            out=pm16, in0=dmax, scalar1=gmax_ps[:, 0:1], scalar2=None,
            op0=OP.is_equal,
        )
        js_ps = psum.tile([1, 1], F32, tag="dmy")
        nc.tensor.matmul(
            js_ps[:, :], pm16[:, 0:1], JIDX16[:, 0:1], start=True, stop=True
        )
        with nc.allow_low_precision(reason="exact small-int index value"):
            nc.vector.tensor_copy(j32, js_ps[0:1, 0:1])
        jv = nc.tensor.value_load(j32[0:1, 0:1])
        nc.vector.tensor_scalar(
            out=MASK, in0=DEN, scalar1=gmax_ps[:, 0:1], scalar2=None, op0=OP.is_equal
        )
        # w1 = mask * r, r = sqrt(C)*q  (the +eps in r is a 5e-6 rel
        # perturbation of the output row; dropped)
        nc.vector.scalar_tensor_tensor(
            out=W1, in0=MASK, scalar=float(np.sqrt(C)), in1=QQ,
            op0=OP.mult, op1=OP.mult,
        )
        nc.vector.reduce_sum(out=w1sel, in_=W1, axis=AX.X)
        nc.vector.tensor_copy(w1sel16, w1sel)
        # w2 = sum_j W1*MU (only the winner row survives the mask)
        nc.vector.scalar_tensor_tensor(
            out=scrj, in0=W1, scalar=1.0, in1=MU,
            op0=OP.mult, op1=OP.mult, accum_out=w1mu,
        )
        nc.vector.tensor_scalar_mul(nw1mu, w1mu, -1.0)
        nc.vector.tensor_copy(nw1mu16, nw1mu)

        # ---- Phase 4: center row = r*(x_j* - mu_j*) ----
        cc_ps = psum.tile([1, C], F32, tag="mx")
        nc.tensor.matmul(
            cc_ps[:, :], w1sel16[:, 0:1], xb16[:, bass.ds(jv, 1), :],
            start=True, stop=False,
        )
        nc.tensor.matmul(
            cc_ps[:, :], nw1mu16[:, 0:1], ones16C[:, :],
            start=False, stop=True,
        )
        nc.scalar.copy(out=cen16, in_=cc_ps[0:1, :])

        # ---- Phase 5: out = relu(proj_w @ center + proj_b) via pwT ----
        # PSUM bf16 writes must be 4B aligned: put the two halves at
        # element offsets 0 and 2 of a [P, 4] tile.
        ccol_ps = psum.tile([P, 4], BF16, tag="cen")
        nc.tensor.transpose(ccol_ps[:, 0:1], cen16[0:1, 0:P], Id16[0:1, 0:1])
        nc.tensor.transpose(ccol_ps[:, 2:3], cen16[0:1, P:C], Id16[0:1, 0:1])
        nc.vector.tensor_copy(cencol[:, 0:1], ccol_ps[:, 0:1])
        nc.vector.tensor_copy(cencol[:, 1:2], ccol_ps[:, 2:3])
        o_ps = psum.tile([1, CR], F32, tag="cen")
        nc.tensor.matmul(
            o_ps[:, :], cencol[:, 0:1], pwT0[:, :], start=True, stop=False
        )
        nc.tensor.matmul(
            o_ps[:, :], cencol[:, 1:2], pwT1[:, :], start=False, stop=True
        )
        nc.vector.tensor_add(o_row, o_ps[0:1, :], pb_row[0:1, :])
        nc.vector.tensor_scalar_max(out=o_row, in0=o_row, scalar1=0.0)
        nc.sync.dma_start(out=out_d[None, :], in_=o_row)

    return nc


def _get_nc() -> bass.Bass:
    if "nc" not in _CACHE:
        _CACHE["nc"] = _build_nc()
    return _CACHE["nc"]


def _ensure_ntff_hook():
    """The image's antenv package lacks axon_hooks; shim it so
    run_bass_kernel_spmd(trace=True) can reach the NTFF profiler."""
    import types

    if "antenv.axon_hooks" in sys.modules:
        return
    m = types.ModuleType("antenv.axon_hooks")
    _hook = [None]
    m.set_axon_ntff_profile_hook = lambda h: _hook.__setitem__(0, h)
    m.get_axon_ntff_profile_hook = lambda: _hook[0]
    sys.modules["antenv.axon_hooks"] = m
    try:
        import antenv

        antenv.axon_hooks = m
        from trn_agent_boot.trn_boot import _ntff_profile_via_ctypes

        m.set_axon_ntff_profile_hook(
            _ntff_profile_via_ctypes("/opt/axon/libaxon_pjrt.so")
        )
    except Exception:
        pass


def _run(x, proj_w, proj_b, trace=False):
    if trace:
        _ensure_ntff_hook()
    nc = _get_nc()
    in_maps = [
        {
            "x": np.ascontiguousarray(x[b], dtype=np.float32),
            "proj_w": np.ascontiguousarray(proj_w, dtype=np.float32),
            "proj_b": np.ascontiguousarray(proj_b, dtype=np.float32),
        }
        for b in range(B)
    ]
    res = run_bass_kernel_spmd(nc, in_maps, list(range(B)), trace=trace)
    out = np.stack([res.results[b]["out"].reshape(1, CR) for b in range(B)])
    return out.astype(np.float32), res


def kernel(x, ln_w, ln_b, proj_w, proj_b):
    x = np.asarray(x)
    ln_w = np.asarray(ln_w)
    ln_b = np.asarray(ln_b)
    proj_w = np.asarray(proj_w)
    proj_b = np.asarray(proj_b)
    if not (np.allclose(ln_w, 1.0) and np.allclose(ln_b, 0.0)):
        # General ln_w/ln_b fallback (never hit with the spec's fills: ones/zeros).
        return _kernel_numpy(x, ln_w, ln_b, proj_w, proj_b)
    out, _ = _run(x, proj_w, proj_b, trace=False)
    return out


def _kernel_numpy(x, ln_w, ln_b, proj_w, proj_b):
    x = x.astype(np.float32)
    mu = x.mean(-1, keepdims=True)
    var = x.var(-1, keepdims=True)
    xn = (x - mu) / np.sqrt(var + LN_EPS) * ln_w + ln_b
    nrm = np.linalg.norm(xn, axis=-1, keepdims=True)
    out = []
    for b in range(x.shape[0]):
        cos = (xn[b] @ xn[b].T) / (nrm[b] @ nrm[b].T + 1e-8)
        den = cos.sum(-1)
        mask = (den == den.max()).astype(np.float32)[:, None]
        center = (xn[b] * mask).sum(0)
        out.append(np.maximum(proj_w @ center + proj_b, 0.0))
    return np.stack(out)[:, None, :].astype(np.float32)


# revision 31
# speedup vs baseline: 1.1086x; 1.0455x over previous
"""Trainium2 Bass kernel for nn_CCG_46273977647541.

Reference pipeline per batch (B=8 -> one NeuronCore each, no cross-core
communication): LayerNorm -> NxN cosine similarity -> density row-sum ->
argmax row as cluster center -> 256->64 projection + relu.

The NxN similarity is never materialized.  With ln_w==1, ln_b==0 (the
spec's deterministic fills) the density factorizes exactly through the
CENTERED rows xc_n = x_n - mu_n:

  u_n       = xc_n / |xc_n|,   |xc_n| = sqrt(C*var_n)
  density_n = u_n . sum_m u_m = q_n * (xc_n . S),  S = sum_m q_m xc_m
  q_n       = rsqrt(C*var_n)

Centering cancels out of the dots entirely: with t1 = sum_m q_m x_m over
the RAW rows,

  density_n = q_n * (x_n . t1  -  mu_n * sum(t1))

because xc_n . 1 = 0.  So the kernel never materializes centered data:

  DMA   x is loaded f32->bf16 with the cast done INLINE by the SDMA
        engines (SWDGE dtype-cast path, nc.gpsimd.dma_start).  This
        removes the entire per-tile cast pass from the compute engines;
        SBUF holds only the 2MB bf16 copy.
  DVE   paired-tile bn_stats ([P,2,256] -> [P,2,6]) + half-merges -> mu,
        var, q; then most of the 32 dot tiles (STT+accum vs broadcast t1).
  ACT   sqrt's; the remaining dot tiles via the square expansion
        x.t1 = (sum(x+t1)^2 - (C var + C mu^2) - sum t1^2)/2 over
        PE-built z=x+t1 PSUM pairs.
  PE    warmup chain (HAM clock-gate release), t1 matmuls (lhsT = q
        column, rhs = raw bf16 tile), z pairs, argmax transposes,
        center gather (+ mu correction via a ones-tile matmul), and the
        projection against a pre-transposed proj_w.

The center row is x-hat_j* = r_j* (x_j* - mu_j*) with r ~= sqrt(C)*q
(the +eps inside r is a 5e-6 relative perturbation; dropped).  The mu
correction of the gather is folded into a second accumulating matmul
with an all-ones rhs tile.

Numerics: bf16 data/matmuls with f32 accumulation, f32 stats and q.
Measured density error vs exact f32 on the spec inputs ~0.06 against a
minimum top-2 gap of 0.26; end-to-end relative error ~2e-3 (gate 2e-2).

Infrastructure notes: this walrus build accepts only ONE semaphore wait
per engine instruction and rejects some custom ISA ops; _split_multi_waits
post-processes the BIR JSON to hoist extra waits onto EventSemaphore
carriers and neutralize non-fatal SeqAsserts.
"""

import sys

sys.path.insert(0, "/opt/trn_rl_repo")

from contextlib import ExitStack

import numpy as np

import concourse.bass as bass
import concourse.tile as tile
from concourse import mybir
from concourse.bass_utils import run_bass_kernel_spmd
from concourse.tile import add_dep_helper

F32 = mybir.dt.float32
BF16 = mybir.dt.bfloat16
AX = mybir.AxisListType
OP = mybir.AluOpType
ACT = mybir.ActivationFunctionType


def _split_multi_waits(bir_json: bytes) -> bytes:
    """This walrus build accepts at most one semaphore wait per engine
    instruction.  Tile can emit several; hoist all but the last onto
    dedicated EventSemaphore carriers placed immediately before the
    instruction (same engine stream, so semantics are preserved --
    the block order is a topological order of the dep graph)."""
    import json as _json

    bir = _json.loads(bir_json)
    n = 0
    for fn in bir["functions"]:
        for bb in fn["blocks"]:
            new = []
            for inst in bb["instructions"]:
                if inst.get("op_name") == "SeqAssert":
                    inst = {
                        "debug": inst.get("debug", 0),
                        "engine": inst["engine"],
                        "ins": [],
                        "outs": [],
                        "name": inst["name"],
                        "opcode": "EventSemaphore",
                        "sync_info": inst.get("sync_info")
                        or {"on_update": [], "on_wait": []},
                    }
                si = inst.get("sync_info")
                waits = (si or {}).get("on_wait") or []
                if len(waits) > 1:
                    for w in waits[:-1]:
                        n += 1
                        new.append(
                            {
                                "debug": inst.get("debug", 0),
                                "engine": inst["engine"],
                                "ins": [],
                                "outs": [],
                                "name": f"antsplitw-{n}",
                                "opcode": "EventSemaphore",
                                "sync_info": {"on_update": [], "on_wait": [w]},
                            }
                        )
                    si["on_wait"] = [waits[-1]]
                new.append(inst)
            bb["instructions"] = new
    return _json.dumps(bir).encode()


def _install_wait_splitter():
    from concourse import bass_utils as _bu
    from concourse import bass2jax as _b2j

    if getattr(_bu, "_ant_wait_splitter", False):
        return
    _orig = _bu.compile_bir_kernel

    def _patched(bir_json, tmpdir, neff_name="file.neff"):
        return _orig(_split_multi_waits(bir_json), tmpdir, neff_name)

    _bu.compile_bir_kernel = _patched
    _bu._ant_wait_splitter = True
    if getattr(_b2j, "compile_bir_kernel", None) is _orig:
        _b2j.compile_bir_kernel = _patched


_install_wait_splitter()

B, N, C, CR = 8, 4096, 256, 64
P = 128
NT = N // P  # 32 row tiles per core
LN_EPS = 1e-5

_CACHE: dict = {}


def _build_nc() -> bass.Bass:
    nc = bass.Bass(enable_asserts=False)
    x_d = nc.declare_dram_parameter("x", [N, C], F32, isOutput=False)
    pw_d = nc.declare_dram_parameter("proj_w", [CR, C], F32, isOutput=False)
    pb_d = nc.declare_dram_parameter("proj_b", [CR], F32, isOutput=False)
    out_d = nc.declare_dram_parameter("out", [CR], F32, isOutput=True)

    with ExitStack() as ctx:
        tc = ctx.enter_context(tile.TileContext(nc))
        small = ctx.enter_context(tc.tile_pool(name="small", bufs=1))
        scrp = ctx.enter_context(tc.tile_pool(name="scr", bufs=6))
        psum = ctx.enter_context(tc.tile_pool(name="ps", bufs=1, space="PSUM"))
        zpool = ctx.enter_context(tc.tile_pool(name="z", bufs=2, space="PSUM"))

        # Row n of this core's batch lives at (partition n//NT, tile n%NT):
        # partition-major so each DMA descriptor reads contiguous DRAM.
        xb16 = small.tile([P, NT, C], BF16)
        ST6 = small.tile([P, NT, 6], F32)  # per-tile even/odd half stats
        MSM2 = small.tile([P, NT, 2], F32)
        DD = small.tile([P, NT], F32)
        D2 = small.tile([P, NT], F32)
        DH = small.tile([P, NT], F32)
        M2C = small.tile([P, NT], F32)
        VA = small.tile([P, NT], F32)
        MU = small.tile([P, NT], F32)
        MU2 = small.tile([P, NT], F32)
        CV = small.tile([P, NT], F32)
        QS = small.tile([P, NT], F32)
        QQ = small.tile([P, NT], F32)
        QQb = small.tile([P, NT], BF16)
        XS = small.tile([P, NT], F32)
        XSQ = small.tile([P, NT], F32)
        CORR = small.tile([P, NT], F32)
        CORR2 = small.tile([P, NT], F32)
        TMPD = small.tile([P, NT], F32)
        DEN = small.tile([P, NT], F32)
        MASK = small.tile([P, NT], F32)
        MASKP = small.tile([P, NT], F32)
        W1 = small.tile([P, NT], F32)
        scrj = small.tile([P, NT], F32)
        IOTAJ = small.tile([P, NT], F32)
        ONESN = small.tile([P, NT], F32)
        dmax = small.tile([P, 1], F32)
        JIDX = small.tile([P, 1], F32)
        JIDX16 = small.tile([P, 1], BF16)
        pm16 = small.tile([P, 1], BF16)
        j32 = small.tile([1, 1], mybir.dt.int32)
        gm1 = small.tile([1, 1], F32)
        w1sel = small.tile([P, 1], F32)
        w1sel16 = small.tile([P, 1], BF16)
        w1mu = small.tile([P, 1], F32)
        nw1mu = small.tile([P, 1], F32)
        nw1mu16 = small.tile([P, 1], BF16)
        T1row = small.tile([1, 2 * C], BF16)
        T1b = small.tile([P, C], BF16)
        st1 = small.tile([1, 1], F32)
        nst1 = small.tile([1, 1], F32)
        ssq_scr = small.tile([1, C], BF16)
        ssS1 = small.tile([1, 1], F32)
        s1row = small.tile([1, C], F32)
        pw_sb = small.tile([CR, C], F32)
        pw16 = small.tile([CR, C], BF16)
        pwT0 = small.tile([P, CR], BF16)
        pwT1 = small.tile([P, CR], BF16)
        pb_row = small.tile([1, CR], F32)
        cen16 = small.tile([1, C], BF16)
        cencol = small.tile([P, 2], BF16)
        o_row = small.tile([1, CR], F32)
        warm = small.tile([1, 1], F32)
        ones_sb = small.tile([1, P], F32)
        ones16 = small.tile([1, P], BF16)
        wdum16 = small.tile([1, P], BF16)
        ones16C = small.tile([P, C], BF16)
        id_sb = small.tile([P, P], F32)
        Id16 = small.tile([P, P], BF16)
        onesPf = small.tile([P, P], F32)
        onesP16 = small.tile([P, P], BF16)
        ji32 = small.tile([P, NT], mybir.dt.int32)

        t1_ps = psum.tile([1, C], F32)
        wup_ps = psum.tile([P, P], F32, tag="wup")
        dmy_ps = psum.tile([1, 1], F32, tag="dmy")
        sb_ps = psum.tile([P, C], F32, tag="sb")
        pwt_ps = psum.tile([P, 2 * CR], BF16, tag="cen")

        xv = x_d[:, :].rearrange("(p j) c -> p j c", p=P)

        # ---- x DMA: SWDGE (gpsimd) with the f32->bf16 cast done inline
        # by the SDMA engines; issued before all other Pool work.  The
        # tail chunks shrink to 1 tile: a chunk's data only becomes
        # visible ~1-2us (completion receipt) after its LAST byte, so
        # small late chunks cut the post-load stats latency.
        CBND = [0, 4, 8, 12, 16, 20, 24, 28, 30, 31, 32]
        for c in range(len(CBND) - 1):
            sl = slice(CBND[c], CBND[c + 1])
            nc.gpsimd.dma_start(out=xb16[:, sl, :], in_=xv[:, sl, :])
        # pw/pb on the (idle) SP HWDGE ring
        nc.sync.dma_start(out=pw_sb, in_=pw_d[:, :])
        nc.sync.dma_start(out=pb_row, in_=pb_d[None, :])

        # ---- Constants (DVE; engines otherwise idle pre-load) ----
        nc.vector.memset(warm, 1.0)
        nc.vector.memset(ones_sb, 1.0)
        nc.vector.memset(ones16, 1.0)
        nc.vector.memset(wdum16, 0.0)
        nc.vector.memset(ones16C, 1.0)
        nc.vector.memset(ONESN, 1.0)

        # ACT table load (Sqrt) early
        nc.scalar.activation(out=warm, in_=warm, func=ACT.Sqrt)

        # ---- PE warmup: release the HAM clock gate (~3.4us of sustained
        # activity -> 1.2GHz cold to 2.4GHz warm) before the t1-chain.
        wu = nc.tensor.matmul(
            wup_ps[:, :], ones16[0:1, :], wdum16[0:1, :], start=True, stop=False
        )
        for _ in range(38):
            wu = nc.tensor.matmul(
                wup_ps[:, :], ones16[0:1, :], wdum16[0:1, :], start=False, stop=False
            )
        nc.tensor.matmul(
            wup_ps[:, :], ones16[0:1, :], wdum16[0:1, :], start=False, stop=True
        )

        # ---- Phase 1 ----
        # DVE runs a PURE bn_stats stream (no other DVE work may sit in
        # its in-order queue during the load, or a stalled op blocks all
        # later stats).  Pool does the half-merges and reciprocals for
        # the first groups; the last group's merge/recip run on DVE
        # right after its final bn_stats (lower cross-engine latency).
        # ACT does the sqrt and the f32->bf16 q cast; PE accumulates t1.
        def _merge(eng, sl):
            # mu = (me+mo)/2 ; var = (M2e+M2o)/C + ((me-mo)/2)^2
            eng.tensor_add(MSM2[:, sl, :], ST6[:, sl, 1:3], ST6[:, sl, 4:6])
            eng.tensor_sub(DD[:, sl], ST6[:, sl, 1], ST6[:, sl, 4])
            eng.tensor_scalar_mul(DH[:, sl], DD[:, sl], 0.5)
            eng.tensor_mul(D2[:, sl], DH[:, sl], DH[:, sl])
            eng.tensor_scalar_mul(M2C[:, sl], MSM2[:, sl, 1], 1.0 / C)
            eng.tensor_add(VA[:, sl], M2C[:, sl], D2[:, sl])
            eng.tensor_scalar_mul(MU[:, sl], MSM2[:, sl, 0], 0.5)

        def _merge_dve(sl):
            nc.vector.tensor_add(MSM2[:, sl, :], ST6[:, sl, 1:3], ST6[:, sl, 4:6])
            nc.vector.tensor_sub(DD[:, sl], ST6[:, sl, 1], ST6[:, sl, 4])
            nc.vector.scalar_tensor_tensor(
                out=D2[:, sl], in0=DD[:, sl], scalar=0.25, in1=DD[:, sl],
                op0=OP.mult, op1=OP.mult,
            )
            nc.vector.scalar_tensor_tensor(
                out=VA[:, sl], in0=MSM2[:, sl, 1], scalar=1.0 / C, in1=D2[:, sl],
                op0=OP.mult, op1=OP.add,
            )
            nc.vector.tensor_scalar_mul(MU[:, sl], MSM2[:, sl, 0], 0.5)

        def _t1mm(g0, g1):
            dmy = nc.tensor.matmul(
                dmy_ps[:, :], QQb[:, g0 : g0 + 1], QQb[:, g0 : g0 + 1],
                start=True, stop=True,
            )
            for j in range(g0, g1):
                mm1 = nc.tensor.matmul(
                    t1_ps[:, :], QQb[:, j : j + 1], xb16[:, j, :],
                    start=(j == 0), stop=(j == NT - 1),
                )
                add_dep_helper(mm1.ins, dmy.ins, False, "pe-prejoin")

        NSQ = 11
        GA, GB = 12, 26  # merge groups: [0,GA) Pool, [GA,GB) + [GB,32) DVE
        # PE keep-warm through the load: one tiny matmul gated on each
        # chunk's data (reads one xb16 column) so the HAM clock gate
        # stays released and all later matmuls run at 2.4GHz.  Emitted
        # interleaved with the stats stream so they sit at matching
        # positions in the PE queue.
        # one keep-warm matmul per tile: ~30-60% PE duty through the
        # load keeps the HAM busy-window satisfied continuously
        KW_TILES = set(range(NT))

        def _bn(h):
            nc.vector.bn_stats(out=ST6[:, h, :], in_=xb16[:, h, :])
            if h in KW_TILES:
                nc.tensor.matmul(
                    dmy_ps[:, :], xb16[:, h, 0:1], xb16[:, h, 0:1],
                    start=True, stop=True,
                )

        # DVE: the bn_stats stream with merges/recips spliced into the
        # chunk-receipt wait gaps (each emitted AFTER its producers).
        for h in range(0, GA):
            _bn(h)
        _merge(nc.gpsimd, slice(0, GA))
        nc.scalar.activation(
            out=QS[:, 0:GA], in_=VA[:, 0:GA], func=ACT.Sqrt, scale=float(C)
        )
        for h in range(GA, GA + 6):
            _bn(h)
        nc.vector.reciprocal(out=QQ[:, 0:GA], in_=QS[:, 0:GA])
        nc.scalar.copy(out=QQb[:, 0:GA], in_=QQ[:, 0:GA])
        _t1mm(0, GA)
        for h in range(GA + 6, GB):
            _bn(h)
        _merge_dve(slice(GA, GB))
        nc.scalar.activation(
            out=QS[:, GA:GB], in_=VA[:, GA:GB], func=ACT.Sqrt, scale=float(C)
        )
        for h in range(GB, GB + 2):
            _bn(h)
        nc.vector.reciprocal(out=QQ[:, GA:GB], in_=QS[:, GA:GB])
        nc.scalar.copy(out=QQb[:, GA:GB], in_=QQ[:, GA:GB])
        _t1mm(GA, GB)
        for h in range(GB + 2, NT):
            _bn(h)
        # last group: merge + q on DVE/ACT right behind bn_stats 31
        _merge_dve(slice(GB, NT))
        nc.scalar.activation(
            out=QS[:, GB:NT], in_=VA[:, GB:NT], func=ACT.Sqrt, scale=float(C)
        )
        nc.vector.reciprocal(out=QQ[:, GB:NT], in_=QS[:, GB:NT])
        nc.scalar.copy(out=QQb[:, GB:NT], in_=QQ[:, GB:NT])
        _t1mm(GB, NT)
        # square-path correction (Pool): CORR = (C/2) * (var + mu^2)
        slq = slice(0, NSQ)
        nc.gpsimd.tensor_mul(MU2[:, slq], MU[:, slq], MU[:, slq])
        nc.gpsimd.tensor_add(CV[:, slq], VA[:, slq], MU2[:, slq])
        nc.gpsimd.tensor_scalar_mul(CORR[:, slq], CV[:, slq], float(C) * 0.5)
        # identity matrices + iota column (Pool, needed from the dot
        # phase onward): affine_select picks in_ where j - p == 0
        nc.gpsimd.memset(onesPf, 1.0)
        nc.gpsimd.memset(onesP16, 1.0)
        nc.gpsimd.affine_select(
            out=id_sb, in_=onesPf, pattern=[[1, P]], compare_op=OP.is_equal,
            fill=0.0, base=0, channel_multiplier=-1,
        )
        nc.gpsimd.affine_select(
            out=Id16, in_=onesP16, pattern=[[1, P]], compare_op=OP.is_equal,
            fill=0.0, base=0, channel_multiplier=-1,
        )
        nc.gpsimd.iota(ji32, pattern=[[1, NT]], base=0, channel_multiplier=0)
        nc.gpsimd.tensor_copy(IOTAJ, ji32)

        # ---- pw pre-transpose (idle-time): cast + 2 PE transposes ----
        nc.scalar.copy(out=pw16, in_=pw_sb)
        nc.tensor.transpose(pwt_ps[:, 0:CR], pw16[0:CR, 0:P], Id16[0:CR, 0:CR])
        nc.tensor.transpose(pwt_ps[:, CR : 2 * CR], pw16[0:CR, P:C], Id16[0:CR, 0:CR])

        # ---- t1 finalize + broadcast ----
        nc.scalar.copy(out=T1row[0:1, 0:C], in_=t1_ps[0:1, :])
        nc.tensor.matmul(
            sb_ps[:, :], ones16[0:1, :], T1row[0:1, 0:C], start=True, stop=True
        )
        nc.vector.tensor_copy(T1b, sb_ps[:, :])
        # sum(t1) on ACT (only needed after the dots)
        nc.scalar.activation(
            out=s1row[0:1, :], in_=t1_ps[0:1, :], func=ACT.Identity,
            accum_out=st1[0:1, 0:1],
        )

        # ---- Phase 2: per-row dot x_n . t1 ----
        ssb_ps = psum.tile([P, 1], F32, tag="mx")
        st1b_ps = psum.tile([P, 1], F32, tag="sb")
        for j in range(NSQ, NT):
            scr = scrp.tile([P, C], BF16, tag="scr")
            st = nc.vector.scalar_tensor_tensor(
                out=scr, in0=xb16[:, j, :], scalar=1.0, in1=T1b,
                op0=OP.mult, op1=OP.mult, accum_out=XS[:, j : j + 1],
            )
            if j == NSQ:
                # z-path-only DVE work deferred past dot 0
                nc.vector.tensor_copy(T1row[0:1, C : 2 * C], t1_ps[0:1, :])
                nc.vector.scalar_tensor_tensor(
                    out=ssq_scr, in0=T1row[0:1, 0:C], scalar=1.0,
                    in1=T1row[0:1, 0:C], op0=OP.mult, op1=OP.mult,
                    accum_out=ssS1,
                )
                nc.vector.tensor_scalar_mul(ssS1, ssS1, 0.5)
                nc.vector.tensor_scalar_mul(nst1, st1, -1.0)
            if j == NSQ + 1:
                # consumers emitted AFTER their deferred inputs: z pairs
                # (gating the ACT squares), the sum-t1^2 and -sum(t1)
                # broadcasts
                for k in range(0, NSQ, 2):
                    k1 = min(k + 2, NSQ)
                    zp = zpool.tile([P, 2 * C], F32, tag="z")
                    nc.tensor.matmul(
                        zp[:, 0 : (k1 - k) * C], Id16[:, :],
                        xb16[:, k:k1, :], start=True, stop=False,
                    )
                    nc.tensor.matmul(
                        zp[:, 0 : (k1 - k) * C], ones16[0:1, :],
                        T1row[0:1, 0 : (k1 - k) * C], start=False, stop=True,
                    )
                    for t in range(k1 - k):
                        sqs = scrp.tile([P, C], BF16, tag="sqr")
                        nc.scalar.activation(
                            out=sqs, in_=zp[:, t * C : (t + 1) * C],
                            func=ACT.Square,
                            accum_out=XSQ[:, k + t : k + t + 1],
                        )
                nc.tensor.matmul(
                    ssb_ps[:, :], ones_sb[0:1, :], ssS1[0:1, 0:1],
                    start=True, stop=True,
                )
                nc.tensor.matmul(
                    st1b_ps[:, :], ones_sb[0:1, :], nst1[0:1, 0:1],
                    start=True, stop=True,
                )
            if j == NSQ + 3:
                # fold the 0.5*sum(t1)^2 term into CORR while dots run
                nc.vector.tensor_scalar(
                    out=CORR2[:, 0:NSQ], in0=CORR[:, 0:NSQ],
                    scalar1=ssb_ps[:, 0:1], scalar2=None, op0=OP.add,
                )
            if j == NSQ + 5:
                # pw^T copyback (PSUM -> SBUF), needed only at the proj
                nc.vector.tensor_copy(pwT0, pwt_ps[:, 0:CR])
                nc.vector.tensor_copy(pwT1, pwt_ps[:, CR : 2 * CR])
            if j in (15, 19, 23, 27, 31):
                # keep-warm: a paced dummy matmul (gated on this dot) so
                # the PE never sees a full 3.4us idle window
                kw = nc.tensor.matmul(
                    dmy_ps[:, :], QQb[:, 0:1], QQb[:, 0:1],
                    start=True, stop=True,
                )
                add_dep_helper(kw.ins, st.ins, False, "keepwarm")
        # finalize the square-trick columns: XS = 0.5*XSQ - CORR2
        nc.vector.scalar_tensor_tensor(
            out=XS[:, 0:NSQ], in0=XSQ[:, 0:NSQ], scalar=0.5, in1=CORR2[:, 0:NSQ],
            op0=OP.mult, op1=OP.subtract,
        )

        # density = q * (x.t1 - mu*sum(t1)):  (MU * (-st1)) + XS, then * QQ
        nc.vector.scalar_tensor_tensor(
            out=TMPD, in0=MU, scalar=st1b_ps[:, 0:1], in1=XS,
            op0=OP.mult, op1=OP.add,
        )
        nc.vector.tensor_mul(DEN, TMPD, QQ)

        # ---- Phase 3: global argmax ----
        # Per-partition max + argmax-index; global max via one transpose;
        # j* = <winner-partition one-hot, per-partition argmax index> on PE
        # (avoids any partition-offset reads of the transposed row).
        nc.vector.reduce_max(out=dmax, in_=DEN, axis=AX.X)
        nc.vector.tensor_scalar(
            out=MASKP, in0=DEN, scalar1=dmax[:, 0:1], scalar2=None, op0=OP.is_equal
        )
        nc.vector.scalar_tensor_tensor(
            out=scrj, in0=MASKP, scalar=1.0, in1=IOTAJ,
            op0=OP.mult, op1=OP.mult, accum_out=JIDX,
        )
        nc.vector.tensor_copy(JIDX16, JIDX)
        tr_ps = psum.tile([1, P], F32, tag="wup")
        nc.tensor.transpose(tr_ps[:, :], dmax[:, 0:1], id_sb[:, :])
        nc.vector.reduce_max(out=gm1, in_=tr_ps[0:1, :], axis=AX.X)
        gmax_ps = psum.tile([P, 1], F32, tag="mx")
        nc.tensor.matmul(
            gmax_ps[:, :], ones_sb[0:1, :], gm1[0:1, 0:1], start=True, stop=True
        )
        nc.vector.tensor_scalar(
            out=pm16, in0=dmax, scalar1=gmax_ps[:, 0:1], scalar2=None,
            op0=OP.is_equal,
        )
        js_ps = psum.tile([1, 1], F32, tag="dmy")
        nc.tensor.matmul(
            js_ps[:, :], pm16[:, 0:1], JIDX16[:, 0:1], start=True, stop=True
        )
        with nc.allow_low_precision(reason="exact small-int index value"):
            nc.vector.tensor_copy(j32, js_ps[0:1, 0:1])
        jv = nc.tensor.value_load(j32[0:1, 0:1])
        nc.vector.tensor_scalar(
            out=MASK, in0=DEN, scalar1=gmax_ps[:, 0:1], scalar2=None, op0=OP.is_equal
        )
        # w1 = mask * r, r = sqrt(C)*q  (the +eps in r is a 5e-6 rel
        # perturbation of the output row; dropped)
        nc.vector.scalar_tensor_tensor(
            out=W1, in0=MASK, scalar=float(np.sqrt(C)), in1=QQ,
            op0=OP.mult, op1=OP.mult,
        )
        nc.vector.reduce_sum(out=w1sel, in_=W1, axis=AX.X)
        nc.vector.tensor_copy(w1sel16, w1sel)
        # w2 = sum_j W1*MU (only the winner row survives the mask)
        nc.vector.scalar_tensor_tensor(
            out=scrj, in0=W1, scalar=1.0, in1=MU,
            op0=OP.mult, op1=OP.mult, accum_out=w1mu,
        )
        nc.vector.tensor_scalar_mul(nw1mu, w1mu, -1.0)
        nc.vector.tensor_copy(nw1mu16, nw1mu)

        # ---- Phase 4: center row = r*(x_j* - mu_j*) ----
        cc_ps = psum.tile([1, C], F32, tag="mx")
        nc.tensor.matmul(
            cc_ps[:, :], w1sel16[:, 0:1], xb16[:, bass.ds(jv, 1), :],
            start=True, stop=False,
        )
        nc.tensor.matmul(
            cc_ps[:, :], nw1mu16[:, 0:1], ones16C[:, :],
            start=False, stop=True,
        )
        nc.scalar.copy(out=cen16, in_=cc_ps[0:1, :])

        # ---- Phase 5: out = relu(proj_w @ center + proj_b) via pwT ----
        # PSUM bf16 writes must be 4B aligned: put the two halves at
        # element offsets 0 and 2 of a [P, 4] tile.
        ccol_ps = psum.tile([P, 4], BF16, tag="cen")
        nc.tensor.transpose(ccol_ps[:, 0:1], cen16[0:1, 0:P], Id16[0:1, 0:1])
        nc.tensor.transpose(ccol_ps[:, 2:3], cen16[0:1, P:C], Id16[0:1, 0:1])
        nc.vector.tensor_copy(cencol[:, 0:1], ccol_ps[:, 0:1])
        nc.vector.tensor_copy(cencol[:, 1:2], ccol_ps[:, 2:3])
        o_ps = psum.tile([1, CR], F32, tag="cen")
        nc.tensor.matmul(
            o_ps[:, :], cencol[:, 0:1], pwT0[:, :], start=True, stop=False
        )
        nc.tensor.matmul(
            o_ps[:, :], cencol[:, 1:2], pwT1[:, :], start=False, stop=True
        )
        nc.vector.tensor_add(o_row, o_ps[0:1, :], pb_row[0:1, :])
        nc.vector.tensor_scalar_max(out=o_row, in0=o_row, scalar1=0.0)
        nc.sync.dma_start(out=out_d[None, :], in_=o_row)

    return nc


def _get_nc() -> bass.Bass:
    if "nc" not in _CACHE:
        _CACHE["nc"] = _build_nc()
    return _CACHE["nc"]


def _ensure_ntff_hook():
    """The image's antenv package lacks axon_hooks; shim it so
    run_bass_kernel_spmd(trace=True) can reach the NTFF profiler."""
    import types

    if "antenv.axon_hooks" in sys.modules:
        return
    m = types.ModuleType("antenv.axon_hooks")
    _hook = [None]
    m.set_axon_ntff_profile_hook = lambda h: _hook.__setitem__(0, h)
    m.get_axon_ntff_profile_hook = lambda: _hook[0]
    sys.modules["antenv.axon_hooks"] = m
    try:
        import antenv

        antenv.axon_hooks = m
        from trn_agent_boot.trn_boot import _ntff_profile_via_ctypes

        m.set_axon_ntff_profile_hook(
            _ntff_profile_via_ctypes("/opt/axon/libaxon_pjrt.so")
        )
    except Exception:
        pass


def _run(x, proj_w, proj_b, trace=False):
    if trace:
        _ensure_ntff_hook()
    nc = _get_nc()
    in_maps = [
        {
            "x": np.ascontiguousarray(x[b], dtype=np.float32),
            "proj_w": np.ascontiguousarray(proj_w, dtype=np.float32),
            "proj_b": np.ascontiguousarray(proj_b, dtype=np.float32),
        }
        for b in range(B)
    ]
    res = run_bass_kernel_spmd(nc, in_maps, list(range(B)), trace=trace)
    out = np.stack([res.results[b]["out"].reshape(1, CR) for b in range(B)])
    return out.astype(np.float32), res


def kernel(x, ln_w, ln_b, proj_w, proj_b):
    x = np.asarray(x)
    ln_w = np.asarray(ln_w)
    ln_b = np.asarray(ln_b)
    proj_w = np.asarray(proj_w)
    proj_b = np.asarray(proj_b)
    if not (np.allclose(ln_w, 1.0) and np.allclose(ln_b, 0.0)):
        # General ln_w/ln_b fallback (never hit with the spec's fills: ones/zeros).
        return _kernel_numpy(x, ln_w, ln_b, proj_w, proj_b)
    out, _ = _run(x, proj_w, proj_b, trace=False)
    return out


def _kernel_numpy(x, ln_w, ln_b, proj_w, proj_b):
    x = x.astype(np.float32)
    mu = x.mean(-1, keepdims=True)
    var = x.var(-1, keepdims=True)
    xn = (x - mu) / np.sqrt(var + LN_EPS) * ln_w + ln_b
    nrm = np.linalg.norm(xn, axis=-1, keepdims=True)
    out = []
    for b in range(x.shape[0]):
        cos = (xn[b] @ xn[b].T) / (nrm[b] @ nrm[b].T + 1e-8)
        den = cos.sum(-1)
        mask = (den == den.max()).astype(np.float32)[:, None]
        center = (xn[b] * mask).sum(0)
        out.append(np.maximum(proj_w @ center + proj_b, 0.0))
    return np.stack(out)[:, None, :].astype(np.float32)
